# revision 1
# baseline (speedup 1.0000x reference)
"""Trainium2 Bass kernel for nn_DecoderLayer_65266323030558.

Decoder layer: rmsnorm -> causal self-attn -> rmsnorm -> cross-attn ->
rmsnorm -> top-2-of-24 MoE (sparse: compute only the routed experts).

Sharding (8 cores):
  - Attention: token-parallel. Core c handles batch c//2, T-half c%2.
    Host permutes each core's batch rows so its 256 query tokens are
    always rows 0:256 (uniform SPMD program); the causal mask columns
    are permuted to match and shipped as an additive f32 mask.
  - MoE: expert-parallel (3 experts/core). Normed tokens are AllGathered
    (bf16) along with fp32 router logits; each core compacts the token
    lists for its 3 experts on-device (top-2 + prefix-scan + indirect
    DMA), computes only routed tokens, scatter-adds gated outputs into a
    bf16 accumulator, and a ReduceScatter returns each core its shard.

Matmuls run in bf16 (fp32 PSUM accumulation); norms, softmax, routing,
and residuals stay fp32.
"""
from contextlib import ExitStack

import numpy as np

import concourse.bass as bass
import concourse.mybir as mybir
import concourse.tile as tile
from concourse import bacc
from concourse.bass_utils import run_bass_kernel_spmd
from concourse.masks import make_identity

F32 = mybir.dt.float32
BF16 = mybir.dt.bfloat16
I32 = mybir.dt.int32
Alu = mybir.AluOpType
Act = mybir.ActivationFunctionType
Ax = mybir.AxisListType

B, T, D, H, DH, E, TOPK, HID = 4, 512, 512, 8, 64, 24, 2, 2048
EPS = 1e-6
NCORES = 8
S = 256            # tokens per core
NTOK = B * T       # 2048
CAP = 256          # per-expert token capacity
EPC = E // NCORES  # experts per core = 3
NT = NTOK // 128   # 16 token tiles
KC = D // 128      # 4 contraction chunks over D
HC = HID // 128    # 16 chunks over HID
QH = HID // 512    # 4 quarter-chunks of HID (streaming unit for wg/wu)


def build_program():
    nc = bacc.Bacc(num_devices=NCORES)

    # ---------------- I/O ----------------
    xb = nc.declare_dram_parameter("xb", [T, D], F32, isOutput=False)
    encb = nc.declare_dram_parameter("encb", [T, D], F32, isOutput=False)
    maskadd = nc.declare_dram_parameter("maskadd", [S, T], F32, isOutput=False)
    ebase_in = nc.declare_dram_parameter("ebase", [E, 1], F32, isOutput=False)
    norms_in = nc.declare_dram_parameter("norms", [3, D], F32, isOutput=False)
    router_in = nc.declare_dram_parameter("router_w", [D, E], F32, isOutput=False)
    wattn = {}
    for name in ["sa_wq", "sa_wk", "sa_wv", "sa_wo", "ca_wq", "ca_wk", "ca_wv", "ca_wo"]:
        wattn[name] = nc.declare_dram_parameter(name, [D, D], F32, isOutput=False)
    wg_in = nc.declare_dram_parameter("wg", [EPC, D, HID], F32, isOutput=False)
    wu_in = nc.declare_dram_parameter("wu", [EPC, D, HID], F32, isOutput=False)
    wd_in = nc.declare_dram_parameter("wd", [EPC, HID, D], F32, isOutput=False)
    out_dram = nc.declare_dram_parameter("out", [S, D], F32, isOutput=True)

    # internal DRAM
    xn_sh = nc.dram_tensor("xn_sh", [S, D], BF16)
    lt_sh = nc.dram_tensor("lt_sh", [E, S], F32)
    xn_all = nc.dram_tensor("xn_all", [NTOK, D], BF16, addr_space="Shared")
    lt_all = nc.dram_tensor("lt_all", [NCORES, E, S], F32, addr_space="Shared")
    clist_tt = [nc.dram_tensor(f"clist{tt}", [EPC * CAP, 2], F32) for tt in range(NT)]
    clist_d = nc.dram_tensor("clist_sum", [EPC * CAP, 2], F32)
    accum = nc.dram_tensor("accum", [NTOK, D], BF16)
    rs_out = nc.dram_tensor("rs_out", [S, D], BF16)

    with tile.TileContext(nc) as tc, ExitStack() as ctx:
        # pools opened first get the low SBUF region and are never blocked
        # by later scoped-pool releases.
        const = ctx.enter_context(tc.tile_pool(name="const", bufs=1))
        moew = ctx.enter_context(tc.tile_pool(name="moew", bufs=12))
        wdp = ctx.enter_context(tc.tile_pool(name="wdp", bufs=16))
        wpool = ctx.enter_context(tc.tile_pool(name="wpool", bufs=1))
        pers = ctx.enter_context(tc.tile_pool(name="pers", bufs=1))
        tp = ctx.enter_context(tc.tile_pool(name="tp", bufs=1))
        ps_misc = ctx.enter_context(tc.tile_pool(name="ps_misc", bufs=1, space="PSUM"))

        # ------------- constants -------------
        identb = const.tile([128, 128], BF16)
        make_identity(nc, identb[:])
        identf = const.tile([128, 128], F32)
        make_identity(nc, identf[:])
        ones_f = const.tile([1, 128], F32)
        nc.vector.memset(ones_f[:], 1.0)
        eps_t = const.tile([128, 1], F32)
        nc.vector.memset(eps_t[:], EPS)
        wnb = []
        for i in range(3):
            nrow = const.tile([1, D], F32, tag=f"nrow{i}", name=f"nrow{i}")
            nc.sync.dma_start(out=nrow[:], in_=norms_in[i:i + 1, :])
            pb = ps_misc.tile([128, D], F32, space="PSUM", tag="misc", name=f"wnbp{i}")
            nc.tensor.matmul(out=pb[:], lhsT=ones_f[:], rhs=nrow[:], start=True, stop=True)
            wb = const.tile([128, D], F32, tag=f"wnb{i}", name=f"wnb{i}")
            nc.vector.tensor_copy(out=wb[:], in_=pb[:])
            wnb.append(wb)
        ebase_t = const.tile([E, 1], F32)
        nc.sync.dma_start(out=ebase_t[:], in_=ebase_in[:])
        router_t = const.tile([128, KC * E], F32)
        for kc in range(KC):
            nc.sync.dma_start(out=router_t[:, kc * E:(kc + 1) * E],
                              in_=router_in[kc * 128:(kc + 1) * 128, :])

        # zero-init dram targets early
        zt = const.tile([96, 16], F32)
        nc.vector.memset(zt[:], 0.0)
        for tt in range(NT):
            nc.sync.dma_start(out=clist_tt[tt][:], in_=zt[:])
        zbf = const.tile([128, 512], BF16)
        nc.vector.memset(zbf[:], 0.0)
        for i in range(NT):
            nc.sync.dma_start(out=accum[i * 128:(i + 1) * 128, :], in_=zbf[:])

        # ------------- attention weight loads (cast f32->bf16 in DMA) ----
        wt = {}
        for name in wattn:
            tiles = []
            for kc in range(KC):
                t_ = wpool.tile([128, D], BF16, tag=f"{name}_{kc}", name=f"{name}_{kc}")
                nc.gpsimd.dma_start(out=t_[:], in_=wattn[name][kc * 128:(kc + 1) * 128, :])
                tiles.append(t_)
            wt[name] = tiles

        # ------------- residual x tiles (f32) -------
        xb_t = []
        for i in range(4):
            t_ = pers.tile([128, D], F32, tag=f"xb{i}", name=f"xb{i}")
            nc.sync.dma_start(out=t_[:], in_=xb[i * 128:(i + 1) * 128, :])
            xb_t.append(t_)

        x2 = [None, None]  # filled inside the attention scope

        # ================= phases A-C in a scoped pool =================
        with tc.tile_pool(name="attn", bufs=1) as ap, \
             tc.tile_pool(name="attn_ps", bufs=1, space="PSUM") as aps:

            enc_bf = []
            for i in range(4):
                t_ = ap.tile([128, D], BF16, tag=f"enc{i}", name=f"enc{i}")
                nc.gpsimd.dma_start(out=t_[:], in_=encb[i * 128:(i + 1) * 128, :])
                enc_bf.append(t_)
            mask_t = []
            for i in range(2):
                t_ = ap.tile([128, T], F32, tag=f"mask{i}", name=f"mask{i}")
                nc.sync.dma_start(out=t_[:], in_=maskadd[i * 128:(i + 1) * 128, :])
                mask_t.append(t_)

            def rmsnorm(x_tiles, w_bcast, out_tag, n_tiles, pool, out_dtype=BF16):
                outs = []
                for i in range(n_tiles):
                    sq = ps_misc.tile([128, D], F32, space="PSUM", tag="misc",
                                      name=f"sq_{out_tag}{i}")
                    acc = tp.tile([128, 1], F32, tag="sqacc", bufs=2, name="sqacc")
                    nc.scalar.activation(out=sq[:], in_=x_tiles[i][:], func=Act.Square,
                                         accum_out=acc[:])
                    rms = tp.tile([128, 1], F32, tag="rms", bufs=2, name="rms")
                    nc.scalar.activation(out=rms[:], in_=acc[:], func=Act.Sqrt,
                                         scale=1.0 / D, bias=eps_t[:, :1])
                    rinv = tp.tile([128, 1], F32, tag="rinv", bufs=2, name="rinv")
                    nc.vector.reciprocal(out=rinv[:], in_=rms[:])
                    nt_ = pool.tile([128, D], out_dtype, tag=f"{out_tag}{i}",
                                    name=f"{out_tag}{i}")
                    nc.vector.scalar_tensor_tensor(out=nt_[:], in0=x_tiles[i][:],
                                                   scalar=rinv[:, :1], in1=w_bcast[:],
                                                   op0=Alu.mult, op1=Alu.mult)
                    outs.append(nt_)
                return outs

            def transpose_bf(src_tiles, n_src, out_tag, pool):
                outs = [pool.tile([128, 128 * n_src], BF16, tag=f"{out_tag}{kc}",
                                  name=f"{out_tag}{kc}") for kc in range(KC)]
                for i in range(n_src):
                    for kc in range(KC):
                        pt = aps.tile([128, 128], BF16, space="PSUM", tag="tr",
                                      bufs=2, name="trp")
                        nc.tensor.transpose(out=pt[:],
                                            in_=src_tiles[i][:, kc * 128:(kc + 1) * 128],
                                            identity=identb[:])
                        nc.vector.tensor_copy(out=outs[kc][:, i * 128:(i + 1) * 128],
                                              in_=pt[:])
                return outs

            def attention(qT, kvT, n_keys, wq, wk, wv, wo, masks, resid, out_tag):
                nkt = n_keys // 128
                attn = [ap.tile([128, D], BF16, tag=f"{out_tag}_a{qt}",
                                name=f"{out_tag}_a{qt}") for qt in range(2)]
                for h in range(H):
                    hs = slice(h * DH, (h + 1) * DH)
                    kt_p = aps.tile([DH, n_keys], F32, space="PSUM", tag="kqv", bufs=2,
                                    name="kt_p")
                    for kc in range(KC):
                        nc.tensor.matmul(out=kt_p[:], lhsT=wk[kc][:, hs], rhs=kvT[kc][:],
                                         start=(kc == 0), stop=(kc == KC - 1))
                    kt_s = ap.tile([DH, n_keys], BF16, tag="kt_s", bufs=2, name="kt_s")
                    nc.vector.tensor_copy(out=kt_s[:], in_=kt_p[:])
                    qt_p = aps.tile([DH, S], F32, space="PSUM", tag="kqv", bufs=2,
                                    name="qt_p")
                    for kc in range(KC):
                        nc.tensor.matmul(out=qt_p[:], lhsT=wq[kc][:, hs], rhs=qT[kc],
                                         start=(kc == 0), stop=(kc == KC - 1))
                    qt_s = ap.tile([DH, S], BF16, tag="qt_s", bufs=2, name="qt_s")
                    nc.vector.tensor_copy(out=qt_s[:], in_=qt_p[:])
                    v_s = []
                    for kt in range(nkt):
                        v_p = aps.tile([128, DH], F32, space="PSUM", tag="kqv", bufs=2,
                                       name="v_p")
                        for kc in range(KC):
                            nc.tensor.matmul(out=v_p[:],
                                             lhsT=kvT[kc][:, kt * 128:(kt + 1) * 128],
                                             rhs=wv[kc][:, hs],
                                             start=(kc == 0), stop=(kc == KC - 1))
                        vs = ap.tile([128, DH], BF16, tag=f"v_s{kt}", bufs=2,
                                     name=f"v_s{kt}")
                        nc.vector.tensor_copy(out=vs[:], in_=v_p[:])
                        v_s.append(vs)
                    for qt in range(2):
                        s_p = aps.tile([128, n_keys], F32, space="PSUM", tag="s", bufs=1,
                                       name="s_p")
                        nc.tensor.matmul(out=s_p[:], lhsT=qt_s[:, qt * 128:(qt + 1) * 128],
                                         rhs=kt_s[:], start=True, stop=True)
                        rowsum = tp.tile([128, 1], F32, tag="rowsum", bufs=2,
                                         name="rowsum")
                        p_s = ap.tile([128, n_keys], BF16, tag="p_s", bufs=2, name="p_s")
                        if masks is not None:
                            sm = ap.tile([128, n_keys], F32, tag="sm", bufs=2, name="sm")
                            nc.vector.tensor_tensor(out=sm[:], in0=s_p[:],
                                                    in1=masks[qt][:], op=Alu.add)
                            nc.scalar.activation(out=p_s[:], in_=sm[:], func=Act.Exp,
                                                 scale=DH ** -0.5, accum_out=rowsum[:])
                        else:
                            nc.scalar.activation(out=p_s[:], in_=s_p[:], func=Act.Exp,
                                                 scale=DH ** -0.5, accum_out=rowsum[:])
                        rinv = tp.tile([128, 1], F32, tag="prinv", bufs=2, name="prinv")
                        nc.vector.reciprocal(out=rinv[:], in_=rowsum[:])
                        o_p = aps.tile([128, DH], F32, space="PSUM", tag="o", bufs=1,
                                       name="o_p")
                        for kt in range(nkt):
                            pt = aps.tile([128, 128], BF16, space="PSUM", tag="tr",
                                          bufs=2, name="ptp")
                            nc.tensor.transpose(out=pt[:],
                                                in_=p_s[:, kt * 128:(kt + 1) * 128],
                                                identity=identb[:])
                            pt_s = ap.tile([128, 128], BF16, tag="pt_s", bufs=2,
                                           name="pt_s")
                            nc.vector.tensor_copy(out=pt_s[:], in_=pt[:])
                            nc.tensor.matmul(out=o_p[:], lhsT=pt_s[:], rhs=v_s[kt][:],
                                             start=(kt == 0), stop=(kt == nkt - 1))
                        nc.vector.tensor_scalar_mul(out=attn[qt][:, hs], in0=o_p[:],
                                                    scalar1=rinv[:, :1])
                attnT = transpose_bf(attn, 2, f"{out_tag}_aT", ap)
                outs = []
                for qt in range(2):
                    pr = ps_misc.tile([128, D], F32, space="PSUM", tag="misc",
                                      name="proj")
                    for kc in range(KC):
                        nc.tensor.matmul(out=pr[:],
                                         lhsT=attnT[kc][:, qt * 128:(qt + 1) * 128],
                                         rhs=wo[kc][:], start=(kc == 0),
                                         stop=(kc == KC - 1))
                    xo = pers.tile([128, D], F32, tag=f"{out_tag}_x{qt}",
                                   name=f"{out_tag}_x{qt}")
                    nc.vector.tensor_tensor(out=xo[:], in0=resid[qt][:], in1=pr[:],
                                            op=Alu.add)
                    outs.append(xo)
                return outs

            # phase A: norm1 + self-attention
            n1 = rmsnorm(xb_t, wnb[0], "n1", 4, ap)
            n1T = transpose_bf(n1, 4, "n1T", ap)
            qT_self = [n1T[kc][:, 0:S] for kc in range(KC)]
            x1 = attention(qT_self, n1T, T, wt["sa_wq"], wt["sa_wk"], wt["sa_wv"],
                           wt["sa_wo"], mask_t, xb_t, "sa")

            # phase B: norm2 + cross-attention
            n2 = rmsnorm(x1, wnb[1], "n2", 2, ap)
            n2T = transpose_bf(n2, 2, "n2T", ap)
            encT = transpose_bf(enc_bf, 4, "encT", ap)
            qT_cross = [n2T[kc][:, 0:S] for kc in range(KC)]
            x2_l = attention(qT_cross, encT, T, wt["ca_wq"], wt["ca_wk"], wt["ca_wv"],
                             wt["ca_wo"], None, x1, "ca")
            x2[0], x2[1] = x2_l[0], x2_l[1]

            # phase C: norm3 + router logits + send shards
            n3f = rmsnorm(x2, wnb[2], "n3f", 2, ap, out_dtype=F32)
            for i in range(2):
                nbf = pers.tile([128, D], BF16, tag=f"n3b{i}", name=f"n3b{i}")
                nc.vector.tensor_copy(out=nbf[:], in_=n3f[i][:])
                nc.sync.dma_start(out=xn_sh[i * 128:(i + 1) * 128, :], in_=nbf[:])
            n3T = [ap.tile([128, S], F32, tag=f"n3T{kc}", name=f"n3T{kc}")
                   for kc in range(KC)]
            for i in range(2):
                for kc in range(KC):
                    pt = aps.tile([128, 128], F32, space="PSUM", tag="tr", bufs=2,
                                  name="n3tp")
                    nc.tensor.transpose(out=pt[:], in_=n3f[i][:, kc * 128:(kc + 1) * 128],
                                        identity=identf[:])
                    nc.vector.tensor_copy(out=n3T[kc][:, i * 128:(i + 1) * 128],
                                          in_=pt[:])
            lt_p = ps_misc.tile([E, S], F32, space="PSUM", tag="misc", name="lt_p")
            for kc in range(KC):
                nc.tensor.matmul(out=lt_p[:], lhsT=router_t[:, kc * E:(kc + 1) * E],
                                 rhs=n3T[kc][:], start=(kc == 0), stop=(kc == KC - 1))
            lt_s = tp.tile([E, S], F32, tag="lt_s", name="lt_s")
            nc.vector.tensor_copy(out=lt_s[:], in_=lt_p[:])
            nc.sync.dma_start(out=lt_sh[:], in_=lt_s[:])

        # ================= allgather =================
        grp = [list(range(NCORES))]
        nc.gpsimd.collective_compute("AllGather", Alu.bypass, replica_groups=grp,
                                     ins=[xn_sh[:].opt()], outs=[xn_all[:].opt()])
        nc.gpsimd.collective_compute("AllGather", Alu.bypass, replica_groups=grp,
                                     ins=[lt_sh[:].opt()], outs=[lt_all[:].opt()])

        # ================= phases D-F in a second scope =================
        with tc.tile_pool(name="moe", bufs=1) as mp, \
             tc.tile_pool(name="moe_ps", bufs=1, space="PSUM") as mps:

            # ---- routing ----
            bufA = mp.tile([E, NTOK], F32, tag="bufA", name="bufA")  # LT, later ovf
            bufB = mp.tile([E, NTOK], F32, tag="bufB", name="bufB")  # maskT
            bufC = mp.tile([E, NTOK], F32, tag="bufC", name="bufC")  # cum/excl/offs
            nc.sync.dma_start(out=bufA[:].rearrange("e (c j) -> e c j", c=NCORES),
                              in_=lt_all[:].rearrange("c e j -> e c j"))
            L_all = mp.tile([128, NT * E], F32, tag="L_all", name="L_all")
            for tt in range(NT):
                pt = mps.tile([128, E], F32, space="PSUM", tag="tr2", bufs=2, name="ltp")
                nc.tensor.transpose(out=pt[:], in_=bufA[:, tt * 128:(tt + 1) * 128],
                                    identity=identf[:E, :E])
                nc.vector.tensor_copy(out=L_all[:, tt * E:(tt + 1) * E], in_=pt[:])
            L3 = L_all[:].rearrange("p (t e) -> p t e", e=E)
            m1 = tp.tile([128, NT], F32, tag="m1", name="m1")
            nc.vector.tensor_reduce(out=m1[:], in_=L3, axis=Ax.X, op=Alu.max)
            mask1 = mp.tile([128, NT * E], F32, tag="mask1", name="mask1")
            nc.vector.tensor_tensor(out=mask1[:].rearrange("p (t e) -> p t e", e=E),
                                    in0=L3, in1=m1[:].to_broadcast([128, NT, E]),
                                    op=Alu.is_equal)
            L2 = mp.tile([128, NT * E], F32, tag="L2", name="L2")
            nc.vector.scalar_tensor_tensor(out=L2[:], in0=mask1[:], scalar=-1e30,
                                           in1=L_all[:], op0=Alu.mult, op1=Alu.add)
            m2 = tp.tile([128, NT], F32, tag="m2", name="m2")
            nc.vector.tensor_reduce(out=m2[:],
                                    in_=L2[:].rearrange("p (t e) -> p t e", e=E),
                                    axis=Ax.X, op=Alu.max)
            mask2 = mp.tile([128, NT * E], F32, tag="mask2", name="mask2")
            nc.vector.tensor_tensor(out=mask2[:].rearrange("p (t e) -> p t e", e=E),
                                    in0=L2[:].rearrange("p (t e) -> p t e", e=E),
                                    in1=m2[:].to_broadcast([128, NT, E]),
                                    op=Alu.is_equal)
            d_ = tp.tile([128, NT], F32, tag="d_", name="d_")
            nc.vector.tensor_tensor(out=d_[:], in0=m2[:], in1=m1[:], op=Alu.subtract)
            ed = tp.tile([128, NT], F32, tag="ed", name="ed")
            nc.scalar.activation(out=ed[:], in_=d_[:], func=Act.Exp)
            den = tp.tile([128, NT], F32, tag="den", name="den")
            nc.vector.tensor_scalar_add(out=den[:], in0=ed[:], scalar1=1.0)
            g1 = tp.tile([128, NT], F32, tag="g1", name="g1")
            nc.vector.reciprocal(out=g1[:], in_=den[:])
            g2 = tp.tile([128, NT], F32, tag="g2", name="g2")
            nc.vector.tensor_tensor(out=g2[:], in0=ed[:], in1=g1[:], op=Alu.mult)
            # expert-major mask -> scan -> slot offsets
            mask12 = L2  # L2 no longer needed; reuse its buffer
            nc.vector.tensor_tensor(out=mask12[:], in0=mask1[:], in1=mask2[:],
                                    op=Alu.add)
            for tt in range(NT):
                pt = mps.tile([E, 128], F32, space="PSUM", tag="tr2", bufs=2, name="mtp")
                nc.tensor.transpose(out=pt[:], in_=mask12[:, tt * E:(tt + 1) * E],
                                    identity=identf[:])
                nc.vector.tensor_copy(out=bufB[:, tt * 128:(tt + 1) * 128], in_=pt[:])
            nc.vector.tensor_tensor_scan(out=bufC[:], data0=bufB[:], data1=bufB[:],
                                         initial=0.0, op0=Alu.add, op1=Alu.bypass)
            nc.vector.tensor_tensor(out=bufC[:], in0=bufC[:], in1=bufB[:],
                                    op=Alu.subtract)
            nc.vector.tensor_scalar(out=bufA[:], in0=bufC[:], scalar1=float(CAP) - 0.5,
                                    scalar2=None, op0=Alu.is_gt)
            nc.vector.tensor_scalar_add(out=bufC[:], in0=bufC[:], scalar1=ebase_t[:, :1])
            nc.vector.scalar_tensor_tensor(out=bufC[:], in0=bufA[:], scalar=1e6,
                                           in1=bufC[:], op0=Alu.mult, op1=Alu.add)
            offT = mp.tile([128, NT * E], F32, tag="offT", name="offT")
            for tt in range(NT):
                pt = mps.tile([128, E], F32, space="PSUM", tag="tr2", bufs=2, name="otp")
                nc.tensor.transpose(out=pt[:], in_=bufC[:, tt * 128:(tt + 1) * 128],
                                    identity=identf[:E, :E])
                nc.vector.tensor_copy(out=offT[:, tt * E:(tt + 1) * E], in_=pt[:])
            ids_all = tp.tile([128, NT], F32, tag="ids_all", name="ids_all")
            nc.gpsimd.iota(out=ids_all[:], pattern=[[128, NT]], base=0,
                           channel_multiplier=1, allow_small_or_imprecise_dtypes=True)
            offsel = []
            for k, mk in enumerate([mask1, mask2]):
                w_ = mp.tile([128, NT * E], F32, tag="wsel", name="wsel")
                nc.vector.tensor_tensor(out=w_[:], in0=offT[:], in1=mk[:], op=Alu.mult)
                o_ = tp.tile([128, NT], F32, tag=f"offsel{k}", name=f"offsel{k}")
                nc.vector.tensor_reduce(out=o_[:],
                                        in_=w_[:].rearrange("p (t e) -> p t e", e=E),
                                        axis=Ax.X, op=Alu.add)
                oi = tp.tile([128, NT], I32, tag=f"offi{k}", name=f"offi{k}")
                nc.vector.tensor_copy(out=oi[:], in_=o_[:])
                offsel.append(oi)
            pay = []
            for k, gk in enumerate([g1, g2]):
                p_ = tp.tile([128, NT * 2], F32, tag=f"pay{k}", name=f"pay{k}")
                p3 = p_[:].rearrange("p (t c) -> p t c", c=2)
                nc.vector.tensor_copy(out=p3[:, :, 0:1], in_=ids_all[:].unsqueeze(2))
                nc.vector.tensor_copy(out=p3[:, :, 1:2], in_=gk[:].unsqueeze(2))
                pay.append(p_)
            for tt in range(NT):
                for k in range(2):
                    nc.gpsimd.indirect_dma_start(
                        out=clist_tt[tt][:],
                        out_offset=bass.IndirectOffsetOnAxis(
                            ap=offsel[k][:, tt:tt + 1], axis=0),
                        in_=pay[k][:, tt * 2:tt * 2 + 2],
                        in_offset=None,
                        bounds_check=EPC * CAP - 1, oob_is_err=False)
            cl_sum = tp.tile([128, 12], F32, tag="cl_sum", name="cl_sum")
            cl_ld = []
            for tt in range(NT):
                t_ = tp.tile([128, 12], F32, tag=f"cl_ld{tt}", name=f"cl_ld{tt}")
                nc.sync.dma_start(out=t_[:],
                                  in_=clist_tt[tt][:].rearrange("a b -> (a b)")
                                  .rearrange("(p f) -> p f", p=128))
                cl_ld.append(t_)
            nc.vector.tensor_tensor(out=cl_sum[:], in0=cl_ld[0][:], in1=cl_ld[1][:],
                                    op=Alu.add)
            for tt in range(2, NT):
                nc.vector.tensor_tensor(out=cl_sum[:], in0=cl_sum[:], in1=cl_ld[tt][:],
                                        op=Alu.add)
            nc.sync.dma_start(out=clist_d[:].rearrange("a b -> (a b)")
                              .rearrange("(p f) -> p f", p=128), in_=cl_sum[:])

            # ---- expert compute ----
            for e in range(EPC):
                ids_i = []
                gates_i = []
                xeT = [mp.tile([128, CAP], BF16, tag=f"xeT_{kc}", bufs=2,
                               name=f"xeT{e}_{kc}") for kc in range(KC)]
                for ct in range(2):
                    cl = tp.tile([128, 2], F32, tag=f"cl{e}_{ct}", name=f"cl{e}_{ct}")
                    row0 = e * CAP + ct * 128
                    nc.sync.dma_start(out=cl[:], in_=clist_d[row0:row0 + 128, :])
                    ids = tp.tile([128, 1], I32, tag=f"ids{e}_{ct}", name=f"ids{e}_{ct}")
                    nc.vector.tensor_copy(out=ids[:], in_=cl[:, 0:1])
                    ids_i.append(ids)
                    gates_i.append(cl)
                    xe = mp.tile([128, D], BF16, tag="xe", bufs=2, name=f"xe{e}_{ct}")
                    nc.gpsimd.indirect_dma_start(
                        out=xe[:], out_offset=None, in_=xn_all[:],
                        in_offset=bass.IndirectOffsetOnAxis(ap=ids[:, :1], axis=0))
                    for kc in range(KC):
                        pt = mps.tile([128, 128], BF16, space="PSUM", tag="tr2",
                                      bufs=2, name="xetp")
                        nc.tensor.transpose(out=pt[:],
                                            in_=xe[:, kc * 128:(kc + 1) * 128],
                                            identity=identb[:])
                        nc.vector.tensor_copy(out=xeT[kc][:, ct * 128:(ct + 1) * 128],
                                              in_=pt[:])
                wd_e = []
                for hc in range(HC):
                    t_ = wdp.tile([128, D], BF16, tag="wd", name=f"wd{e}_{hc}")
                    nc.gpsimd.dma_start(out=t_[:],
                                        in_=wd_in[e, hc * 128:(hc + 1) * 128, :])
                    wd_e.append(t_)
                aT = []
                for q in range(QH):
                    wg_q, wu_q = [], []
                    for kc in range(KC):
                        tg = moew.tile([128, 512], BF16, tag="wgq",
                                       name=f"wg{e}_{q}_{kc}")
                        nc.gpsimd.dma_start(
                            out=tg[:], in_=wg_in[e, kc * 128:(kc + 1) * 128,
                                                 q * 512:(q + 1) * 512])
                        wg_q.append(tg)
                        tu = moew.tile([128, 512], BF16, tag="wuq",
                                       name=f"wu{e}_{q}_{kc}")
                        nc.gpsimd.dma_start(
                            out=tu[:], in_=wu_in[e, kc * 128:(kc + 1) * 128,
                                                 q * 512:(q + 1) * 512])
                        wu_q.append(tu)
                    for hcl in range(4):
                        hsl = slice(hcl * 128, (hcl + 1) * 128)
                        hT = mps.tile([128, CAP], F32, space="PSUM", tag="hu", bufs=3,
                                      name="hT")
                        for kc in range(KC):
                            nc.tensor.matmul(out=hT[:], lhsT=wg_q[kc][:, hsl],
                                             rhs=xeT[kc][:],
                                             start=(kc == 0), stop=(kc == KC - 1))
                        uT = mps.tile([128, CAP], F32, space="PSUM", tag="hu", bufs=3,
                                      name="uT")
                        for kc in range(KC):
                            nc.tensor.matmul(out=uT[:], lhsT=wu_q[kc][:, hsl],
                                             rhs=xeT[kc][:],
                                             start=(kc == 0), stop=(kc == KC - 1))
                        sl = tp.tile([128, CAP], BF16, tag="silu", bufs=2, name="silu")
                        nc.scalar.activation(out=sl[:], in_=hT[:], func=Act.Silu)
                        a_ = mp.tile([128, CAP], BF16, tag="aT", bufs=20,
                                     name=f"aT{e}_{q}_{hcl}")
                        nc.vector.tensor_tensor(out=a_[:], in0=sl[:], in1=uT[:],
                                                op=Alu.mult)
                        aT.append(a_)
                for ct in range(2):
                    y_p = mps.tile([128, D], F32, space="PSUM", tag="y", bufs=1,
                                   name="y_p")
                    for hc in range(HC):
                        nc.tensor.matmul(out=y_p[:],
                                         lhsT=aT[hc][:, ct * 128:(ct + 1) * 128],
                                         rhs=wd_e[hc][:], start=(hc == 0),
                                         stop=(hc == HC - 1))
                    y_g = tp.tile([128, D], BF16, tag="y_g", bufs=2, name=f"y_g{e}_{ct}")
                    nc.vector.tensor_scalar_mul(out=y_g[:], in0=y_p[:],
                                                scalar1=gates_i[ct][:, 1:2])
                    nc.gpsimd.indirect_dma_start(
                        out=accum[:], out_offset=bass.IndirectOffsetOnAxis(
                            ap=ids_i[ct][:, :1], axis=0),
                        in_=y_g[:], in_offset=None, compute_op=Alu.add)

            # ---- reduce-scatter + residual ----
            nc.gpsimd.collective_compute("ReduceScatter", Alu.add, replica_groups=grp,
                                         ins=[accum[:].opt()], outs=[rs_out[:].opt()])
            for qt in range(2):
                rs_t = tp.tile([128, D], BF16, tag=f"rs{qt}", name=f"rs{qt}")
                nc.sync.dma_start(out=rs_t[:], in_=rs_out[qt * 128:(qt + 1) * 128, :])
                o_t = tp.tile([128, D], F32, tag=f"ofin{qt}", name=f"ofin{qt}")
                nc.vector.tensor_tensor(out=o_t[:], in0=x2[qt][:], in1=rs_t[:],
                                        op=Alu.add)
                nc.sync.dma_start(out=out_dram[qt * 128:(qt + 1) * 128, :], in_=o_t[:])

    nc.compile()
    return nc


_NC_CACHE = None


def _get_program():
    global _NC_CACHE
    if _NC_CACHE is None:
        _NC_CACHE = build_program()
    return _NC_CACHE


def make_in_maps(x, enc_out, causal_mask, norm1_w, norm2_w, norm3_w,
                 sa_wq, sa_wk, sa_wv, sa_wo, ca_wq, ca_wk, ca_wv, ca_wo,
                 router_w, moe_wg, moe_wu, moe_wd):
    x = np.asarray(x, np.float32)
    enc_out = np.asarray(enc_out, np.float32)
    causal_mask = np.asarray(causal_mask)
    fullmask = np.where(causal_mask, np.float32(-1e30), np.float32(0.0))
    norms = np.stack([np.asarray(norm1_w, np.float32),
                      np.asarray(norm2_w, np.float32),
                      np.asarray(norm3_w, np.float32)], 0)
    shared = {
        "norms": norms,
        "router_w": np.asarray(router_w, np.float32),
        "sa_wq": np.asarray(sa_wq, np.float32), "sa_wk": np.asarray(sa_wk, np.float32),
        "sa_wv": np.asarray(sa_wv, np.float32), "sa_wo": np.asarray(sa_wo, np.float32),
        "ca_wq": np.asarray(ca_wq, np.float32), "ca_wk": np.asarray(ca_wk, np.float32),
        "ca_wv": np.asarray(ca_wv, np.float32), "ca_wo": np.asarray(ca_wo, np.float32),
    }
    moe_wg = np.asarray(moe_wg, np.float32)
    moe_wu = np.asarray(moe_wu, np.float32)
    moe_wd = np.asarray(moe_wd, np.float32)

    in_maps = []
    for c in range(NCORES):
        b, h = c // 2, c % 2
        perm = np.concatenate([np.arange(h * S, (h + 1) * S),
                               np.arange((1 - h) * S, (2 - h) * S)])
        xb_perm = x[b][perm]
        mrows = fullmask[h * S:(h + 1) * S][:, perm]
        ebase = np.full((E, 1), 1e6, np.float32)
        for i in range(EPC):
            ebase[EPC * c + i, 0] = i * CAP
        m = dict(shared)
        m["xb"] = np.ascontiguousarray(xb_perm)
        m["encb"] = np.ascontiguousarray(enc_out[b])
        m["maskadd"] = np.ascontiguousarray(mrows)
        m["ebase"] = ebase
        m["wg"] = np.ascontiguousarray(moe_wg[EPC * c:EPC * (c + 1)])
        m["wu"] = np.ascontiguousarray(moe_wu[EPC * c:EPC * (c + 1)])
        m["wd"] = np.ascontiguousarray(moe_wd[EPC * c:EPC * (c + 1)])
        in_maps.append(m)
    return in_maps


def assemble_out(results):
    out = np.empty((B, T, D), np.float32)
    for c in range(NCORES):
        b, h = c // 2, c % 2
        out[b, h * S:(h + 1) * S] = results[c]["out"]
    return out


def kernel(**inputs):
    nc = _get_program()
    in_maps = make_in_maps(**inputs)
    res = run_bass_kernel_spmd(nc, in_maps, list(range(NCORES)))
    return assemble_out(res.results)


if __name__ == "__main__":
    import reference
    inp = reference.setup_inputs()
    got = kernel(**{k: np.asarray(v) for k, v in inp.items()})
    exp = np.asarray(reference.reference(**inp))
    err = np.abs(got - exp)
    print("abs max err:", err.max(), "rel:", err.max() / np.abs(exp).max())



# revision 7
# speedup vs baseline: 1.0090x; 1.0090x over previous
"""Trainium2 Bass kernel for nn_DecoderLayer_65266323030558.

Decoder layer: rmsnorm -> causal self-attn -> rmsnorm -> cross-attn ->
rmsnorm -> top-2-of-24 MoE (sparse: compute only the routed experts).

Sharding (8 cores):
  - Attention: token-parallel. Core c handles batch c//2, T-half c%2.
    Host permutes each core's batch rows so its 256 query tokens are
    always rows 0:256 (uniform SPMD program); the causal mask columns
    are permuted to match and shipped as an additive f32 mask.
  - MoE: expert-parallel (3 experts/core). Normed tokens are AllGathered
    (bf16) along with fp32 router logits; each core compacts the token
    lists for its 3 experts on-device (top-2 + prefix-scan + indirect
    DMA), computes only routed tokens, scatter-adds gated outputs into a
    bf16 accumulator, and a ReduceScatter returns each core its shard.

Matmuls run in bf16 (fp32 PSUM accumulation); norms, softmax, routing,
and residuals stay fp32.
"""
from contextlib import ExitStack

import numpy as np

import concourse.bass as bass
import concourse.mybir as mybir
import concourse.tile as tile
from concourse import bacc
from concourse.bass_utils import run_bass_kernel_spmd
from concourse.masks import make_identity

F32 = mybir.dt.float32
BF16 = mybir.dt.bfloat16
I32 = mybir.dt.int32
Alu = mybir.AluOpType
Act = mybir.ActivationFunctionType
Ax = mybir.AxisListType

B, T, D, H, DH, E, TOPK, HID = 4, 512, 512, 8, 64, 24, 2, 2048
EPS = 1e-6
NCORES = 8
S = 256            # tokens per core
NTOK = B * T       # 2048
CAP = 256          # per-expert token capacity
EPC = E // NCORES  # experts per core = 3
NT = NTOK // 128   # 16 token tiles
KC = D // 128      # 4 contraction chunks over D
HC = HID // 128    # 16 chunks over HID
QH = HID // 512    # 4 quarter-chunks of HID (streaming unit for wg/wu)


def build_program():
    nc = bacc.Bacc(num_devices=NCORES)

    # ---------------- I/O ----------------
    xb = nc.declare_dram_parameter("xb", [T, D], F32, isOutput=False)
    encb = nc.declare_dram_parameter("encb", [T, D], F32, isOutput=False)
    maskadd = nc.declare_dram_parameter("maskadd", [S, T], F32, isOutput=False)
    ebase_in = nc.declare_dram_parameter("ebase", [E, 1], F32, isOutput=False)
    norms_in = nc.declare_dram_parameter("norms", [3, D], F32, isOutput=False)
    router_in = nc.declare_dram_parameter("router_w", [D, E], F32, isOutput=False)
    wattn = {}
    for name in ["sa_wq", "sa_wk", "sa_wv", "sa_wo", "ca_wq", "ca_wk", "ca_wv", "ca_wo"]:
        wattn[name] = nc.declare_dram_parameter(name, [D, D], F32, isOutput=False)
    wg_in = nc.declare_dram_parameter("wg", [EPC, D, HID], F32, isOutput=False)
    wu_in = nc.declare_dram_parameter("wu", [EPC, D, HID], F32, isOutput=False)
    wd_in = nc.declare_dram_parameter("wd", [EPC, HID, D], F32, isOutput=False)
    out_dram = nc.declare_dram_parameter("out", [S, D], F32, isOutput=True)

    # internal DRAM
    xn_sh = nc.dram_tensor("xn_sh", [S, D], BF16)
    lt_sh = nc.dram_tensor("lt_sh", [E, S], F32)
    xn_all = nc.dram_tensor("xn_all", [NTOK, D], BF16, addr_space="Shared")
    lt_all = nc.dram_tensor("lt_all", [NCORES, E, S], F32, addr_space="Shared")
    clist_tt = [nc.dram_tensor(f"clist{tt}", [EPC * CAP, 2], F32) for tt in range(NT)]
    clist_d = nc.dram_tensor("clist_sum", [EPC * CAP, 2], F32)
    accum = nc.dram_tensor("accum", [NTOK, D], BF16)
    rs_out = nc.dram_tensor("rs_out", [S, D], BF16)

    with tile.TileContext(nc) as tc, ExitStack() as ctx:
        # pools opened first get the low SBUF region and are never blocked
        # by later scoped-pool releases.
        const = ctx.enter_context(tc.tile_pool(name="const", bufs=1))
        moew = ctx.enter_context(tc.tile_pool(name="moew", bufs=12))
        wdp = ctx.enter_context(tc.tile_pool(name="wdp", bufs=16))
        wpool = ctx.enter_context(tc.tile_pool(name="wpool", bufs=1))
        pers = ctx.enter_context(tc.tile_pool(name="pers", bufs=1))
        tp = ctx.enter_context(tc.tile_pool(name="tp", bufs=1))
        ps_misc = ctx.enter_context(tc.tile_pool(name="ps_misc", bufs=1, space="PSUM"))

        # ------------- constants -------------
        identb = const.tile([128, 128], BF16)
        make_identity(nc, identb[:])
        identf = const.tile([128, 128], F32)
        make_identity(nc, identf[:])
        ones_f = const.tile([1, 128], F32)
        nc.vector.memset(ones_f[:], 1.0)
        eps_t = const.tile([128, 1], F32)
        nc.vector.memset(eps_t[:], EPS)
        wnb = []
        for i in range(3):
            nrow = const.tile([1, D], F32, tag=f"nrow{i}", name=f"nrow{i}")
            nc.sync.dma_start(out=nrow[:], in_=norms_in[i:i + 1, :])
            pb = ps_misc.tile([128, D], F32, space="PSUM", tag="misc", name=f"wnbp{i}")
            nc.tensor.matmul(out=pb[:], lhsT=ones_f[:], rhs=nrow[:], start=True, stop=True)
            wb = const.tile([128, D], F32, tag=f"wnb{i}", name=f"wnb{i}")
            nc.vector.tensor_copy(out=wb[:], in_=pb[:])
            wnb.append(wb)
        ebase_t = const.tile([E, 1], F32)
        nc.sync.dma_start(out=ebase_t[:], in_=ebase_in[:])
        router_t = const.tile([128, KC * E], F32)
        for kc in range(KC):
            nc.sync.dma_start(out=router_t[:, kc * E:(kc + 1) * E],
                              in_=router_in[kc * 128:(kc + 1) * 128, :])

        # zero-init dram targets early
        zt = const.tile([96, 16], F32)
        nc.vector.memset(zt[:], 0.0)
        for tt in range(NT):
            nc.sync.dma_start(out=clist_tt[tt][:], in_=zt[:])
        zbf = const.tile([128, 512], BF16)
        nc.vector.memset(zbf[:], 0.0)
        for i in range(NT):
            nc.sync.dma_start(out=accum[i * 128:(i + 1) * 128, :], in_=zbf[:])

        # ------------- attention weight loads (cast f32->bf16 in DMA) ----
        wt = {}
        for name in wattn:
            tiles = []
            for kc in range(KC):
                t_ = wpool.tile([128, D], BF16, tag=f"{name}_{kc}", name=f"{name}_{kc}")
                nc.gpsimd.dma_start(out=t_[:], in_=wattn[name][kc * 128:(kc + 1) * 128, :])
                tiles.append(t_)
            wt[name] = tiles

        # ------------- residual x tiles (f32) -------
        xb_t = []
        for i in range(4):
            t_ = pers.tile([128, D], F32, tag=f"xb{i}", name=f"xb{i}")
            nc.sync.dma_start(out=t_[:], in_=xb[i * 128:(i + 1) * 128, :])
            xb_t.append(t_)

        x2 = [None, None]  # filled inside the attention scope

        # ================= phases A-C in a scoped pool =================
        with tc.tile_pool(name="attn", bufs=1) as ap, \
             tc.tile_pool(name="attn_ps", bufs=1, space="PSUM") as aps:

            enc_bf = []
            for i in range(4):
                t_ = ap.tile([128, D], BF16, tag=f"enc{i}", name=f"enc{i}")
                nc.gpsimd.dma_start(out=t_[:], in_=encb[i * 128:(i + 1) * 128, :])
                enc_bf.append(t_)
            mask_t = []
            for i in range(2):
                t_ = ap.tile([128, T], F32, tag=f"mask{i}", name=f"mask{i}")
                nc.sync.dma_start(out=t_[:], in_=maskadd[i * 128:(i + 1) * 128, :])
                mask_t.append(t_)

            def rmsnorm(x_tiles, w_bcast, out_tag, n_tiles, pool, out_dtype=BF16):
                outs = []
                for i in range(n_tiles):
                    sq = ps_misc.tile([128, D], F32, space="PSUM", tag="misc",
                                      name=f"sq_{out_tag}{i}")
                    acc = tp.tile([128, 1], F32, tag="sqacc", bufs=2, name="sqacc")
                    nc.scalar.activation(out=sq[:], in_=x_tiles[i][:], func=Act.Square,
                                         accum_out=acc[:])
                    rms = tp.tile([128, 1], F32, tag="rms", bufs=2, name="rms")
                    nc.scalar.activation(out=rms[:], in_=acc[:], func=Act.Sqrt,
                                         scale=1.0 / D, bias=eps_t[:, :1])
                    rinv = tp.tile([128, 1], F32, tag="rinv", bufs=2, name="rinv")
                    nc.vector.reciprocal(out=rinv[:], in_=rms[:])
                    nt_ = pool.tile([128, D], out_dtype, tag=f"{out_tag}{i}",
                                    name=f"{out_tag}{i}")
                    nc.vector.scalar_tensor_tensor(out=nt_[:], in0=x_tiles[i][:],
                                                   scalar=rinv[:, :1], in1=w_bcast[:],
                                                   op0=Alu.mult, op1=Alu.mult)
                    outs.append(nt_)
                return outs

            def transpose_bf(src_tiles, n_src, out_tag, pool):
                outs = [pool.tile([128, 128 * n_src], BF16, tag=f"{out_tag}{kc}",
                                  name=f"{out_tag}{kc}") for kc in range(KC)]
                for i in range(n_src):
                    for kc in range(KC):
                        pt = aps.tile([128, 128], BF16, space="PSUM", tag="tr",
                                      bufs=2, name="trp")
                        nc.tensor.transpose(out=pt[:],
                                            in_=src_tiles[i][:, kc * 128:(kc + 1) * 128],
                                            identity=identb[:])
                        nc.vector.tensor_copy(out=outs[kc][:, i * 128:(i + 1) * 128],
                                              in_=pt[:])
                return outs

            def attention(qT, kvT, n_keys, wq, wk, wv, wo, masks, resid, out_tag):
                nkt = n_keys // 128
                attn = [ap.tile([128, D], BF16, tag=f"{out_tag}_a{qt}",
                                name=f"{out_tag}_a{qt}") for qt in range(2)]
                for h in range(H):
                    hs = slice(h * DH, (h + 1) * DH)
                    kt_p = aps.tile([DH, n_keys], F32, space="PSUM", tag="kqv", bufs=2,
                                    name="kt_p")
                    for kc in range(KC):
                        nc.tensor.matmul(out=kt_p[:], lhsT=wk[kc][:, hs], rhs=kvT[kc][:],
                                         start=(kc == 0), stop=(kc == KC - 1))
                    kt_s = ap.tile([DH, n_keys], BF16, tag="kt_s", bufs=2, name="kt_s")
                    nc.vector.tensor_copy(out=kt_s[:], in_=kt_p[:])
                    qt_p = aps.tile([DH, S], F32, space="PSUM", tag="kqv", bufs=2,
                                    name="qt_p")
                    for kc in range(KC):
                        nc.tensor.matmul(out=qt_p[:], lhsT=wq[kc][:, hs], rhs=qT[kc],
                                         start=(kc == 0), stop=(kc == KC - 1))
                    qt_s = ap.tile([DH, S], BF16, tag="qt_s", bufs=2, name="qt_s")
                    nc.vector.tensor_copy(out=qt_s[:], in_=qt_p[:])
                    v_s = []
                    for kt in range(nkt):
                        v_p = aps.tile([128, DH], F32, space="PSUM", tag="kqv", bufs=2,
                                       name="v_p")
                        for kc in range(KC):
                            nc.tensor.matmul(out=v_p[:],
                                             lhsT=kvT[kc][:, kt * 128:(kt + 1) * 128],
                                             rhs=wv[kc][:, hs],
                                             start=(kc == 0), stop=(kc == KC - 1))
                        vs = ap.tile([128, DH], BF16, tag=f"v_s{kt}", bufs=2,
                                     name=f"v_s{kt}")
                        nc.vector.tensor_copy(out=vs[:], in_=v_p[:])
                        v_s.append(vs)
                    for qt in range(2):
                        s_p = aps.tile([128, n_keys], F32, space="PSUM", tag="s", bufs=1,
                                       name="s_p")
                        nc.tensor.matmul(out=s_p[:], lhsT=qt_s[:, qt * 128:(qt + 1) * 128],
                                         rhs=kt_s[:], start=True, stop=True)
                        rowsum = tp.tile([128, 1], F32, tag="rowsum", bufs=2,
                                         name="rowsum")
                        p_s = ap.tile([128, n_keys], BF16, tag="p_s", bufs=2, name="p_s")
                        if masks is not None:
                            sm = ap.tile([128, n_keys], F32, tag="sm", bufs=2, name="sm")
                            nc.vector.tensor_tensor(out=sm[:], in0=s_p[:],
                                                    in1=masks[qt][:], op=Alu.add)
                            nc.scalar.activation(out=p_s[:], in_=sm[:], func=Act.Exp,
                                                 scale=DH ** -0.5, accum_out=rowsum[:])
                        else:
                            nc.scalar.activation(out=p_s[:], in_=s_p[:], func=Act.Exp,
                                                 scale=DH ** -0.5, accum_out=rowsum[:])
                        rinv = tp.tile([128, 1], F32, tag="prinv", bufs=2, name="prinv")
                        nc.vector.reciprocal(out=rinv[:], in_=rowsum[:])
                        o_p = aps.tile([128, DH], F32, space="PSUM", tag="o", bufs=1,
                                       name="o_p")
                        for kt in range(nkt):
                            pt = aps.tile([128, 128], BF16, space="PSUM", tag="tr",
                                          bufs=2, name="ptp")
                            nc.tensor.transpose(out=pt[:],
                                                in_=p_s[:, kt * 128:(kt + 1) * 128],
                                                identity=identb[:])
                            pt_s = ap.tile([128, 128], BF16, tag="pt_s", bufs=2,
                                           name="pt_s")
                            nc.vector.tensor_copy(out=pt_s[:], in_=pt[:])
                            nc.tensor.matmul(out=o_p[:], lhsT=pt_s[:], rhs=v_s[kt][:],
                                             start=(kt == 0), stop=(kt == nkt - 1))
                        nc.vector.tensor_scalar_mul(out=attn[qt][:, hs], in0=o_p[:],
                                                    scalar1=rinv[:, :1])
                attnT = transpose_bf(attn, 2, f"{out_tag}_aT", ap)
                outs = []
                for qt in range(2):
                    pr = ps_misc.tile([128, D], F32, space="PSUM", tag="misc",
                                      name="proj")
                    for kc in range(KC):
                        nc.tensor.matmul(out=pr[:],
                                         lhsT=attnT[kc][:, qt * 128:(qt + 1) * 128],
                                         rhs=wo[kc][:], start=(kc == 0),
                                         stop=(kc == KC - 1))
                    xo = pers.tile([128, D], F32, tag=f"{out_tag}_x{qt}",
                                   name=f"{out_tag}_x{qt}")
                    nc.vector.tensor_tensor(out=xo[:], in0=resid[qt][:], in1=pr[:],
                                            op=Alu.add)
                    outs.append(xo)
                return outs

            # phase A: norm1 + self-attention
            with nc.named_scope("A_self"):
                n1 = rmsnorm(xb_t, wnb[0], "n1", 4, ap)
                n1T = transpose_bf(n1, 4, "n1T", ap)
                qT_self = [n1T[kc][:, 0:S] for kc in range(KC)]
                x1 = attention(qT_self, n1T, T, wt["sa_wq"], wt["sa_wk"], wt["sa_wv"],
                               wt["sa_wo"], mask_t, xb_t, "sa")

            # phase B: norm2 + cross-attention
            with nc.named_scope("B_cross"):
                n2 = rmsnorm(x1, wnb[1], "n2", 2, ap)
                n2T = transpose_bf(n2, 2, "n2T", ap)
                encT = transpose_bf(enc_bf, 4, "encT", ap)
                qT_cross = [n2T[kc][:, 0:S] for kc in range(KC)]
                x2_l = attention(qT_cross, encT, T, wt["ca_wq"], wt["ca_wk"],
                                 wt["ca_wv"], wt["ca_wo"], None, x1, "ca")
                x2[0], x2[1] = x2_l[0], x2_l[1]

            # phase C: norm3 + router logits + send shards
            n3f = rmsnorm(x2, wnb[2], "n3f", 2, ap, out_dtype=F32)
            for i in range(2):
                nbf = pers.tile([128, D], BF16, tag=f"n3b{i}", name=f"n3b{i}")
                nc.vector.tensor_copy(out=nbf[:], in_=n3f[i][:])
                nc.sync.dma_start(out=xn_sh[i * 128:(i + 1) * 128, :], in_=nbf[:])
            n3T = [ap.tile([128, S], F32, tag=f"n3T{kc}", name=f"n3T{kc}")
                   for kc in range(KC)]
            for i in range(2):
                for kc in range(KC):
                    pt = aps.tile([128, 128], F32, space="PSUM", tag="tr", bufs=2,
                                  name="n3tp")
                    nc.tensor.transpose(out=pt[:], in_=n3f[i][:, kc * 128:(kc + 1) * 128],
                                        identity=identf[:])
                    nc.vector.tensor_copy(out=n3T[kc][:, i * 128:(i + 1) * 128],
                                          in_=pt[:])
            lt_p = ps_misc.tile([E, S], F32, space="PSUM", tag="misc", name="lt_p")
            for kc in range(KC):
                nc.tensor.matmul(out=lt_p[:], lhsT=router_t[:, kc * E:(kc + 1) * E],
                                 rhs=n3T[kc][:], start=(kc == 0), stop=(kc == KC - 1))
            lt_s = tp.tile([E, S], F32, tag="lt_s", name="lt_s")
            nc.vector.tensor_copy(out=lt_s[:], in_=lt_p[:])
            nc.sync.dma_start(out=lt_sh[:], in_=lt_s[:])

        # ================= allgather =================
        grp = [list(range(NCORES))]
        with nc.named_scope("AG"):
            nc.gpsimd.collective_compute("AllGather", Alu.bypass, replica_groups=grp,
                                         ins=[xn_sh[:].opt()], outs=[xn_all[:].opt()])
            nc.gpsimd.collective_compute("AllGather", Alu.bypass, replica_groups=grp,
                                         ins=[lt_sh[:].opt()], outs=[lt_all[:].opt()])

        # ================= phases D-F in a second scope =================
        with tc.tile_pool(name="moe", bufs=1) as mp, \
             tc.tile_pool(name="moe_ps", bufs=1, space="PSUM") as mps:

            # ---- routing ----
            rt_ctx = nc.named_scope("Routing")
            rt_ctx.__enter__()
            bufA = mp.tile([E, NTOK], F32, tag="bufA", name="bufA")  # LT, later ovf
            bufB = mp.tile([E, NTOK], F32, tag="bufB", name="bufB")  # maskT
            bufC = mp.tile([E, NTOK], F32, tag="bufC", name="bufC")  # cum/excl/offs
            nc.sync.dma_start(out=bufA[:].rearrange("e (c j) -> e c j", c=NCORES),
                              in_=lt_all[:].rearrange("c e j -> e c j"))
            L_all = mp.tile([128, NT * E], F32, tag="L_all", name="L_all")
            for tt in range(NT):
                pt = mps.tile([128, E], F32, space="PSUM", tag="tr2", bufs=2, name="ltp")
                nc.tensor.transpose(out=pt[:], in_=bufA[:, tt * 128:(tt + 1) * 128],
                                    identity=identf[:E, :E])
                nc.vector.tensor_copy(out=L_all[:, tt * E:(tt + 1) * E], in_=pt[:])
            L3 = L_all[:].rearrange("p (t e) -> p t e", e=E)
            m1 = tp.tile([128, NT], F32, tag="m1", name="m1")
            nc.vector.tensor_reduce(out=m1[:], in_=L3, axis=Ax.X, op=Alu.max)
            mask1 = mp.tile([128, NT * E], F32, tag="mask1", name="mask1")
            nc.vector.tensor_tensor(out=mask1[:].rearrange("p (t e) -> p t e", e=E),
                                    in0=L3, in1=m1[:].to_broadcast([128, NT, E]),
                                    op=Alu.is_equal)
            L2 = mp.tile([128, NT * E], F32, tag="L2", name="L2")
            nc.vector.scalar_tensor_tensor(out=L2[:], in0=mask1[:], scalar=-1e30,
                                           in1=L_all[:], op0=Alu.mult, op1=Alu.add)
            m2 = tp.tile([128, NT], F32, tag="m2", name="m2")
            nc.vector.tensor_reduce(out=m2[:],
                                    in_=L2[:].rearrange("p (t e) -> p t e", e=E),
                                    axis=Ax.X, op=Alu.max)
            mask2 = mp.tile([128, NT * E], F32, tag="mask2", name="mask2")
            nc.vector.tensor_tensor(out=mask2[:].rearrange("p (t e) -> p t e", e=E),
                                    in0=L2[:].rearrange("p (t e) -> p t e", e=E),
                                    in1=m2[:].to_broadcast([128, NT, E]),
                                    op=Alu.is_equal)
            d_ = tp.tile([128, NT], F32, tag="d_", name="d_")
            nc.vector.tensor_tensor(out=d_[:], in0=m2[:], in1=m1[:], op=Alu.subtract)
            ed = tp.tile([128, NT], F32, tag="ed", name="ed")
            nc.scalar.activation(out=ed[:], in_=d_[:], func=Act.Exp)
            den = tp.tile([128, NT], F32, tag="den", name="den")
            nc.vector.tensor_scalar_add(out=den[:], in0=ed[:], scalar1=1.0)
            g1 = tp.tile([128, NT], F32, tag="g1", name="g1")
            nc.vector.reciprocal(out=g1[:], in_=den[:])
            g2 = tp.tile([128, NT], F32, tag="g2", name="g2")
            nc.vector.tensor_tensor(out=g2[:], in0=ed[:], in1=g1[:], op=Alu.mult)
            # expert-major mask -> scan -> slot offsets
            mask12 = L2  # L2 no longer needed; reuse its buffer
            nc.vector.tensor_tensor(out=mask12[:], in0=mask1[:], in1=mask2[:],
                                    op=Alu.add)
            for tt in range(NT):
                pt = mps.tile([E, 128], F32, space="PSUM", tag="tr2", bufs=2, name="mtp")
                nc.tensor.transpose(out=pt[:], in_=mask12[:, tt * E:(tt + 1) * E],
                                    identity=identf[:])
                nc.vector.tensor_copy(out=bufB[:, tt * 128:(tt + 1) * 128], in_=pt[:])
            nc.vector.tensor_tensor_scan(out=bufC[:], data0=bufB[:], data1=bufB[:],
                                         initial=0.0, op0=Alu.add, op1=Alu.bypass)
            nc.vector.tensor_tensor(out=bufC[:], in0=bufC[:], in1=bufB[:],
                                    op=Alu.subtract)
            nc.vector.tensor_scalar(out=bufA[:], in0=bufC[:], scalar1=float(CAP) - 0.5,
                                    scalar2=None, op0=Alu.is_gt)
            nc.vector.tensor_scalar_add(out=bufC[:], in0=bufC[:], scalar1=ebase_t[:, :1])
            nc.vector.scalar_tensor_tensor(out=bufC[:], in0=bufA[:], scalar=1e6,
                                           in1=bufC[:], op0=Alu.mult, op1=Alu.add)
            offT = mp.tile([128, NT * E], F32, tag="offT", name="offT")
            for tt in range(NT):
                pt = mps.tile([128, E], F32, space="PSUM", tag="tr2", bufs=2, name="otp")
                nc.tensor.transpose(out=pt[:], in_=bufC[:, tt * 128:(tt + 1) * 128],
                                    identity=identf[:E, :E])
                nc.vector.tensor_copy(out=offT[:, tt * E:(tt + 1) * E], in_=pt[:])
            ids_all = tp.tile([128, NT], F32, tag="ids_all", name="ids_all")
            nc.gpsimd.iota(out=ids_all[:], pattern=[[128, NT]], base=0,
                           channel_multiplier=1, allow_small_or_imprecise_dtypes=True)
            offsel = []
            for k, mk in enumerate([mask1, mask2]):
                w_ = mp.tile([128, NT * E], F32, tag="wsel", name="wsel")
                nc.vector.tensor_tensor(out=w_[:], in0=offT[:], in1=mk[:], op=Alu.mult)
                o_ = tp.tile([128, NT], F32, tag=f"offsel{k}", name=f"offsel{k}")
                nc.vector.tensor_reduce(out=o_[:],
                                        in_=w_[:].rearrange("p (t e) -> p t e", e=E),
                                        axis=Ax.X, op=Alu.add)
                oi = tp.tile([128, NT], I32, tag=f"offi{k}", name=f"offi{k}")
                nc.vector.tensor_copy(out=oi[:], in_=o_[:])
                offsel.append(oi)
            pay = []
            for k, gk in enumerate([g1, g2]):
                p_ = tp.tile([128, NT * 2], F32, tag=f"pay{k}", name=f"pay{k}")
                p3 = p_[:].rearrange("p (t c) -> p t c", c=2)
                nc.vector.tensor_copy(out=p3[:, :, 0:1], in_=ids_all[:].unsqueeze(2))
                nc.vector.tensor_copy(out=p3[:, :, 1:2], in_=gk[:].unsqueeze(2))
                pay.append(p_)
            for tt in range(NT):
                for k in range(2):
                    nc.gpsimd.indirect_dma_start(
                        out=clist_tt[tt][:],
                        out_offset=bass.IndirectOffsetOnAxis(
                            ap=offsel[k][:, tt:tt + 1], axis=0),
                        in_=pay[k][:, tt * 2:tt * 2 + 2],
                        in_offset=None,
                        bounds_check=EPC * CAP - 1, oob_is_err=False)
            cl_sum = tp.tile([128, 12], F32, tag="cl_sum", name="cl_sum")
            cl_ld = []
            for tt in range(NT):
                t_ = tp.tile([128, 12], F32, tag=f"cl_ld{tt}", name=f"cl_ld{tt}")
                nc.sync.dma_start(out=t_[:],
                                  in_=clist_tt[tt][:].rearrange("a b -> (a b)")
                                  .rearrange("(p f) -> p f", p=128))
                cl_ld.append(t_)
            nc.vector.tensor_tensor(out=cl_sum[:], in0=cl_ld[0][:], in1=cl_ld[1][:],
                                    op=Alu.add)
            for tt in range(2, NT):
                nc.vector.tensor_tensor(out=cl_sum[:], in0=cl_sum[:], in1=cl_ld[tt][:],
                                        op=Alu.add)
            nc.sync.dma_start(out=clist_d[:].rearrange("a b -> (a b)")
                              .rearrange("(p f) -> p f", p=128), in_=cl_sum[:])
            rt_ctx.__exit__(None, None, None)

            # ---- expert compute ----
            for e in range(EPC):
                e_ctx = nc.named_scope(f"Expert{e}")
                e_ctx.__enter__()
                ids_i = []
                gates_i = []
                xeT = [mp.tile([128, CAP], BF16, tag=f"xeT_{kc}", bufs=2,
                               name=f"xeT{e}_{kc}") for kc in range(KC)]
                for ct in range(2):
                    cl = tp.tile([128, 2], F32, tag=f"cl{e}_{ct}", name=f"cl{e}_{ct}")
                    row0 = e * CAP + ct * 128
                    nc.sync.dma_start(out=cl[:], in_=clist_d[row0:row0 + 128, :])
                    ids = tp.tile([128, 1], I32, tag=f"ids{e}_{ct}", name=f"ids{e}_{ct}")
                    nc.vector.tensor_copy(out=ids[:], in_=cl[:, 0:1])
                    ids_i.append(ids)
                    gates_i.append(cl)
                    xe = mp.tile([128, D], BF16, tag="xe", bufs=2, name=f"xe{e}_{ct}")
                    nc.gpsimd.indirect_dma_start(
                        out=xe[:], out_offset=None, in_=xn_all[:],
                        in_offset=bass.IndirectOffsetOnAxis(ap=ids[:, :1], axis=0))
                    for kc in range(KC):
                        pt = mps.tile([128, 128], BF16, space="PSUM", tag="tr2",
                                      bufs=2, name="xetp")
                        nc.tensor.transpose(out=pt[:],
                                            in_=xe[:, kc * 128:(kc + 1) * 128],
                                            identity=identb[:])
                        nc.vector.tensor_copy(out=xeT[kc][:, ct * 128:(ct + 1) * 128],
                                              in_=pt[:])
                wd_e = []
                for hc in range(HC):
                    t_ = wdp.tile([128, D], BF16, tag="wd", name=f"wd{e}_{hc}")
                    nc.gpsimd.dma_start(out=t_[:],
                                        in_=wd_in[e, hc * 128:(hc + 1) * 128, :])
                    wd_e.append(t_)
                aT = []
                for q in range(QH):
                    wg_q, wu_q = [], []
                    for kc in range(KC):
                        tg = moew.tile([128, 512], BF16, tag="wgq",
                                       name=f"wg{e}_{q}_{kc}")
                        nc.gpsimd.dma_start(
                            out=tg[:], in_=wg_in[e, kc * 128:(kc + 1) * 128,
                                                 q * 512:(q + 1) * 512])
                        wg_q.append(tg)
                        tu = moew.tile([128, 512], BF16, tag="wuq",
                                       name=f"wu{e}_{q}_{kc}")
                        nc.gpsimd.dma_start(
                            out=tu[:], in_=wu_in[e, kc * 128:(kc + 1) * 128,
                                                 q * 512:(q + 1) * 512])
                        wu_q.append(tu)
                    for hcl in range(4):
                        hsl = slice(hcl * 128, (hcl + 1) * 128)
                        hT = mps.tile([128, CAP], F32, space="PSUM", tag="hu", bufs=3,
                                      name="hT")
                        for kc in range(KC):
                            nc.tensor.matmul(out=hT[:], lhsT=wg_q[kc][:, hsl],
                                             rhs=xeT[kc][:],
                                             start=(kc == 0), stop=(kc == KC - 1))
                        uT = mps.tile([128, CAP], F32, space="PSUM", tag="hu", bufs=3,
                                      name="uT")
                        for kc in range(KC):
                            nc.tensor.matmul(out=uT[:], lhsT=wu_q[kc][:, hsl],
                                             rhs=xeT[kc][:],
                                             start=(kc == 0), stop=(kc == KC - 1))
                        sl = tp.tile([128, CAP], BF16, tag="silu", bufs=2, name="silu")
                        nc.scalar.activation(out=sl[:], in_=hT[:], func=Act.Silu)
                        a_ = mp.tile([128, CAP], BF16, tag="aT", bufs=20,
                                     name=f"aT{e}_{q}_{hcl}")
                        nc.vector.tensor_tensor(out=a_[:], in0=sl[:], in1=uT[:],
                                                op=Alu.mult)
                        aT.append(a_)
                for ct in range(2):
                    y_p = mps.tile([128, D], F32, space="PSUM", tag="y", bufs=1,
                                   name="y_p")
                    for hc in range(HC):
                        nc.tensor.matmul(out=y_p[:],
                                         lhsT=aT[hc][:, ct * 128:(ct + 1) * 128],
                                         rhs=wd_e[hc][:], start=(hc == 0),
                                         stop=(hc == HC - 1))
                    y_g = tp.tile([128, D], BF16, tag="y_g", bufs=2, name=f"y_g{e}_{ct}")
                    nc.vector.tensor_scalar_mul(out=y_g[:], in0=y_p[:],
                                                scalar1=gates_i[ct][:, 1:2])
                    nc.gpsimd.indirect_dma_start(
                        out=accum[:], out_offset=bass.IndirectOffsetOnAxis(
                            ap=ids_i[ct][:, :1], axis=0),
                        in_=y_g[:], in_offset=None, compute_op=Alu.add)
                e_ctx.__exit__(None, None, None)

            # ---- reduce-scatter + residual ----
            with nc.named_scope("RS"):
                nc.gpsimd.collective_compute("ReduceScatter", Alu.add,
                                             replica_groups=grp,
                                             ins=[accum[:].opt()],
                                             outs=[rs_out[:].opt()])
            for qt in range(2):
                rs_t = tp.tile([128, D], BF16, tag=f"rs{qt}", name=f"rs{qt}")
                nc.sync.dma_start(out=rs_t[:], in_=rs_out[qt * 128:(qt + 1) * 128, :])
                o_t = tp.tile([128, D], F32, tag=f"ofin{qt}", name=f"ofin{qt}")
                nc.vector.tensor_tensor(out=o_t[:], in0=x2[qt][:], in1=rs_t[:],
                                        op=Alu.add)
                nc.sync.dma_start(out=out_dram[qt * 128:(qt + 1) * 128, :], in_=o_t[:])

    nc.compile()
    return nc


_NC_CACHE = None


def _get_program():
    global _NC_CACHE
    if _NC_CACHE is None:
        _NC_CACHE = build_program()
    return _NC_CACHE


def make_in_maps(x, enc_out, causal_mask, norm1_w, norm2_w, norm3_w,
                 sa_wq, sa_wk, sa_wv, sa_wo, ca_wq, ca_wk, ca_wv, ca_wo,
                 router_w, moe_wg, moe_wu, moe_wd):
    x = np.asarray(x, np.float32)
    enc_out = np.asarray(enc_out, np.float32)
    causal_mask = np.asarray(causal_mask)
    fullmask = np.where(causal_mask, np.float32(-1e30), np.float32(0.0))
    norms = np.stack([np.asarray(norm1_w, np.float32),
                      np.asarray(norm2_w, np.float32),
                      np.asarray(norm3_w, np.float32)], 0)
    shared = {
        "norms": norms,
        "router_w": np.asarray(router_w, np.float32),
        "sa_wq": np.asarray(sa_wq, np.float32), "sa_wk": np.asarray(sa_wk, np.float32),
        "sa_wv": np.asarray(sa_wv, np.float32), "sa_wo": np.asarray(sa_wo, np.float32),
        "ca_wq": np.asarray(ca_wq, np.float32), "ca_wk": np.asarray(ca_wk, np.float32),
        "ca_wv": np.asarray(ca_wv, np.float32), "ca_wo": np.asarray(ca_wo, np.float32),
    }
    moe_wg = np.asarray(moe_wg, np.float32)
    moe_wu = np.asarray(moe_wu, np.float32)
    moe_wd = np.asarray(moe_wd, np.float32)

    in_maps = []
    for c in range(NCORES):
        b, h = c // 2, c % 2
        perm = np.concatenate([np.arange(h * S, (h + 1) * S),
                               np.arange((1 - h) * S, (2 - h) * S)])
        xb_perm = x[b][perm]
        mrows = fullmask[h * S:(h + 1) * S][:, perm]
        ebase = np.full((E, 1), 1e6, np.float32)
        for i in range(EPC):
            ebase[EPC * c + i, 0] = i * CAP
        m = dict(shared)
        m["xb"] = np.ascontiguousarray(xb_perm)
        m["encb"] = np.ascontiguousarray(enc_out[b])
        m["maskadd"] = np.ascontiguousarray(mrows)
        m["ebase"] = ebase
        m["wg"] = np.ascontiguousarray(moe_wg[EPC * c:EPC * (c + 1)])
        m["wu"] = np.ascontiguousarray(moe_wu[EPC * c:EPC * (c + 1)])
        m["wd"] = np.ascontiguousarray(moe_wd[EPC * c:EPC * (c + 1)])
        in_maps.append(m)
    return in_maps


def assemble_out(results):
    out = np.empty((B, T, D), np.float32)
    for c in range(NCORES):
        b, h = c // 2, c % 2
        out[b, h * S:(h + 1) * S] = results[c]["out"]
    return out


def kernel(**inputs):
    nc = _get_program()
    in_maps = make_in_maps(**inputs)
    res = run_bass_kernel_spmd(nc, in_maps, list(range(NCORES)))
    return assemble_out(res.results)


if __name__ == "__main__":
    import reference
    inp = reference.setup_inputs()
    got = kernel(**{k: np.asarray(v) for k, v in inp.items()})
    exp = np.asarray(reference.reference(**inp))
    err = np.abs(got - exp)
    print("abs max err:", err.max(), "rel:", err.max() / np.abs(exp).max())



# revision 28
# speedup vs baseline: 1.3325x; 1.3207x over previous
"""Trainium2 Bass kernel for nn_DecoderLayer_65266323030558.

Decoder layer: rmsnorm -> causal self-attn -> rmsnorm -> cross-attn ->
rmsnorm -> top-2-of-24 MoE (sparse: compute only the routed experts).

Sharding (8 cores):
  - Attention: token-parallel. Core c handles batch c//2, T-half c%2.
    Host permutes each core's batch rows so its 256 query tokens are
    always rows 0:256 (uniform SPMD program); the causal mask columns
    are permuted to match and shipped as an additive f32 mask.
  - MoE: expert-parallel (3 experts/core). Normed tokens are AllGathered
    (bf16) along with fp32 router logits; each core compacts the token
    lists for its 3 experts on-device (top-2 + prefix-scan + indirect
    DMA), computes only routed tokens, scatter-adds gated outputs into a
    bf16 accumulator, and a ReduceScatter returns each core its shard.

Matmuls run in bf16 (fp32 PSUM accumulation); norms, softmax, routing,
and residuals stay fp32.
"""
from contextlib import ExitStack

import numpy as np
import ml_dtypes

import concourse.bass as bass
import concourse.mybir as mybir
import concourse.tile as tile
from concourse import bacc, library_config
from concourse.bass_utils import run_bass_kernel_spmd
from concourse.masks import make_identity

F32 = mybir.dt.float32
BF16 = mybir.dt.bfloat16
I32 = mybir.dt.int32
U32 = mybir.dt.uint32
Alu = mybir.AluOpType
Act = mybir.ActivationFunctionType
Ax = mybir.AxisListType

SIM_SAFE_SILU = False  # sim_test sets True: CoreSim lacks Silu

B, T, D, H, DH, E, TOPK, HID = 4, 512, 512, 8, 64, 24, 2, 2048
EPS = 1e-6
NCORES = 8
S = 256            # tokens per core
NTOK = B * T       # 2048
CAP = 256          # per-expert token capacity
EPC = E // NCORES  # experts per core = 3
NT = NTOK // 128   # 16 token tiles
KC = D // 128      # 4 contraction chunks over D
HC = HID // 128    # 16 chunks over HID
QH = HID // 512    # 4 quarter-chunks of HID (streaming unit for wg/wu)


def build_program():
    nc = bacc.Bacc(num_devices=NCORES)

    # ---------------- I/O ----------------
    xb = nc.declare_dram_parameter("xb", [T, D], F32, isOutput=False)
    encb = nc.declare_dram_parameter("encb", [T, D], F32, isOutput=False)
    maskadd = nc.declare_dram_parameter("maskadd", [S, T], F32, isOutput=False)
    eidx_in = nc.declare_dram_parameter("eidx", [16, EPC], I32, isOutput=False)
    gids_in = nc.declare_dram_parameter("gids", [128, 2], F32, isOutput=False)
    sidx_in = nc.declare_dram_parameter("sidx", [128, 2], F32, isOutput=False)
    norms_in = nc.declare_dram_parameter("norms", [3, D], F32, isOutput=False)
    router_in = nc.declare_dram_parameter("router_w", [D, E], F32, isOutput=False)
    wattn = {}
    for name in ["sa_wq", "sa_wk", "sa_wv", "sa_wo", "ca_wq", "ca_wk", "ca_wv", "ca_wo"]:
        wattn[name] = nc.declare_dram_parameter(name, [D, D], F32, isOutput=False)
    wg_in = nc.declare_dram_parameter("wg", [EPC, D, HID], BF16, isOutput=False)
    wu_in = nc.declare_dram_parameter("wu", [EPC, D, HID], BF16, isOutput=False)
    wd_in = nc.declare_dram_parameter("wd", [EPC, HID, D], BF16, isOutput=False)
    out_dram = nc.declare_dram_parameter("out", [S, D], F32, isOutput=True)

    # internal DRAM
    xn_sh = nc.dram_tensor("xn_sh", [S, D], BF16)
    pk_sh = nc.dram_tensor("pk_sh", [E * 2, 128], F32)
    xn_all = nc.dram_tensor("xn_all", [NTOK, D], BF16, addr_space="Shared")
    pk_all = nc.dram_tensor("pk_all", [NCORES * E * 2, 128], F32,
                            addr_space="Shared")
    accum = nc.dram_tensor("accum", [NTOK, D], BF16)
    rs_out = nc.dram_tensor("rs_out", [S, D], BF16)
    sg_scr = [nc.dram_tensor(f"sg_scr{e}", [2, 128], F32) for e in range(EPC)]

    with tile.TileContext(nc) as tc, ExitStack() as ctx:
        # pools opened first get the low SBUF region and are never blocked
        # by later scoped-pool releases.
        const = ctx.enter_context(tc.tile_pool(name="const", bufs=1))
        moew = ctx.enter_context(tc.tile_pool(name="moew", bufs=12))
        wdp = ctx.enter_context(tc.tile_pool(name="wdp", bufs=16))
        wpool = ctx.enter_context(tc.tile_pool(name="wpool", bufs=1))
        pers = ctx.enter_context(tc.tile_pool(name="pers", bufs=1))
        tp = ctx.enter_context(tc.tile_pool(name="tp", bufs=1))
        ps_misc = ctx.enter_context(tc.tile_pool(name="ps_misc", bufs=1, space="PSUM"))

        # ------------- constants -------------
        identb = const.tile([128, 128], BF16)
        make_identity(nc, identb[:])
        identf = const.tile([128, 128], F32)
        make_identity(nc, identf[:])
        ones_f = const.tile([1, 128], F32)
        nc.vector.memset(ones_f[:], 1.0)
        eps_t = const.tile([128, 1], F32)
        nc.vector.memset(eps_t[:], EPS)
        wnb = []
        for i in range(3):
            nrow = const.tile([1, D], F32, tag=f"nrow{i}", name=f"nrow{i}")
            nc.sync.dma_start(out=nrow[:], in_=norms_in[i:i + 1, :])
            pb = ps_misc.tile([128, D], F32, space="PSUM", tag="misc", name=f"wnbp{i}")
            nc.tensor.matmul(out=pb[:], lhsT=ones_f[:], rhs=nrow[:], start=True, stop=True)
            wb = const.tile([128, D], F32, tag=f"wnb{i}", name=f"wnb{i}")
            nc.vector.tensor_copy(out=wb[:], in_=pb[:])
            wnb.append(wb)
        eidx_t = const.tile([16, EPC], I32)
        nc.sync.dma_start(out=eidx_t[:], in_=eidx_in[:])
        gids_t = const.tile([128, 2], F32)
        nc.sync.dma_start(out=gids_t[:], in_=gids_in[:])
        sidx_t = const.tile([128, 2], F32)
        nc.sync.dma_start(out=sidx_t[:], in_=sidx_in[:])
        router_t = const.tile([128, KC * E], F32)
        for kc in range(KC):
            nc.sync.dma_start(out=router_t[:, kc * E:(kc + 1) * E],
                              in_=router_in[kc * 128:(kc + 1) * 128, :])

        # zero-init dram targets early
        zbf = const.tile([128, 512], BF16)
        nc.vector.memset(zbf[:], 0.0)
        for i in range(NT):
            nc.sync.dma_start(out=accum[i * 128:(i + 1) * 128, :], in_=zbf[:])

        # ------------- attention weight loads (cast f32->bf16 in DMA) ----
        wt = {}
        for name in wattn:
            tiles = []
            for kc in range(KC):
                t_ = wpool.tile([128, D], BF16, tag=f"{name}_{kc}", name=f"{name}_{kc}")
                nc.gpsimd.dma_start(out=t_[:], in_=wattn[name][kc * 128:(kc + 1) * 128, :])
                tiles.append(t_)
            wt[name] = tiles

        # ------------- residual x tiles (f32) -------
        xb_t = []
        for i in range(4):
            t_ = pers.tile([128, D], F32, tag=f"xb{i}", name=f"xb{i}")
            nc.sync.dma_start(out=t_[:], in_=xb[i * 128:(i + 1) * 128, :])
            xb_t.append(t_)

        x2 = [None, None]  # filled inside the attention scope

        # ================= phases A-C in a scoped pool =================
        with tc.tile_pool(name="attn", bufs=1) as ap, \
             tc.tile_pool(name="attn_ps", bufs=1, space="PSUM") as aps:

            enc_bf = []
            for i in range(4):
                t_ = ap.tile([128, D], BF16, tag=f"enc{i}", name=f"enc{i}")
                nc.gpsimd.dma_start(out=t_[:], in_=encb[i * 128:(i + 1) * 128, :])
                enc_bf.append(t_)
            mask_t = []
            for i in range(2):
                t_ = ap.tile([128, T], F32, tag=f"mask{i}", name=f"mask{i}")
                nc.sync.dma_start(out=t_[:], in_=maskadd[i * 128:(i + 1) * 128, :])
                mask_t.append(t_)

            def rmsnorm(x_tiles, w_bcast, out_tag, n_tiles, pool, out_dtype=BF16):
                outs = []
                for i in range(n_tiles):
                    sq = ps_misc.tile([128, D], F32, space="PSUM", tag="misc",
                                      name=f"sq_{out_tag}{i}")
                    acc = tp.tile([128, 1], F32, tag="sqacc", bufs=2, name="sqacc")
                    nc.scalar.activation(out=sq[:], in_=x_tiles[i][:], func=Act.Square,
                                         accum_out=acc[:])
                    rms = tp.tile([128, 1], F32, tag="rms", bufs=2, name="rms")
                    nc.scalar.activation(out=rms[:], in_=acc[:], func=Act.Sqrt,
                                         scale=1.0 / D, bias=eps_t[:, :1])
                    rinv = tp.tile([128, 1], F32, tag="rinv", bufs=2, name="rinv")
                    nc.vector.reciprocal(out=rinv[:], in_=rms[:])
                    nt_ = pool.tile([128, D], out_dtype, tag=f"{out_tag}{i}",
                                    name=f"{out_tag}{i}")
                    nc.vector.scalar_tensor_tensor(out=nt_[:], in0=x_tiles[i][:],
                                                   scalar=rinv[:, :1], in1=w_bcast[:],
                                                   op0=Alu.mult, op1=Alu.mult)
                    outs.append(nt_)
                return outs

            def transpose_bf(src_tiles, n_src, out_tag, pool):
                outs = [pool.tile([128, 128 * n_src], BF16, tag=f"{out_tag}{kc}",
                                  name=f"{out_tag}{kc}") for kc in range(KC)]
                for i in range(n_src):
                    for kc in range(KC):
                        pt = aps.tile([128, 128], BF16, space="PSUM", tag="tr",
                                      bufs=2, name="trp")
                        nc.tensor.transpose(out=pt[:],
                                            in_=src_tiles[i][:, kc * 128:(kc + 1) * 128],
                                            identity=identb[:])
                        nc.vector.tensor_copy(out=outs[kc][:, i * 128:(i + 1) * 128],
                                              in_=pt[:])
                return outs

            def attention(qT, kvT, n_keys, wq, wk, wv, wo, masks, resid, out_tag):
                nkt = n_keys // 128
                attn = [ap.tile([128, D], BF16, tag=f"{out_tag}_a{qt}",
                                name=f"{out_tag}_a{qt}") for qt in range(2)]
                for h in range(H):
                    hs = slice(h * DH, (h + 1) * DH)
                    kt_p = aps.tile([DH, n_keys], F32, space="PSUM", tag="kqv", bufs=2,
                                    name="kt_p")
                    for kc in range(KC):
                        nc.tensor.matmul(out=kt_p[:], lhsT=wk[kc][:, hs], rhs=kvT[kc][:],
                                         start=(kc == 0), stop=(kc == KC - 1))
                    kt_s = ap.tile([DH, n_keys], BF16, tag="kt_s", bufs=2, name="kt_s")
                    nc.vector.tensor_copy(out=kt_s[:], in_=kt_p[:])
                    qt_p = aps.tile([DH, S], F32, space="PSUM", tag="kqv", bufs=2,
                                    name="qt_p")
                    for kc in range(KC):
                        nc.tensor.matmul(out=qt_p[:], lhsT=wq[kc][:, hs], rhs=qT[kc],
                                         start=(kc == 0), stop=(kc == KC - 1))
                    qt_s = ap.tile([DH, S], BF16, tag="qt_s", bufs=2, name="qt_s")
                    nc.vector.tensor_copy(out=qt_s[:], in_=qt_p[:])
                    v_s = []
                    for kt in range(nkt):
                        v_p = aps.tile([128, DH], F32, space="PSUM", tag="kqv", bufs=2,
                                       name="v_p")
                        for kc in range(KC):
                            nc.tensor.matmul(out=v_p[:],
                                             lhsT=kvT[kc][:, kt * 128:(kt + 1) * 128],
                                             rhs=wv[kc][:, hs],
                                             start=(kc == 0), stop=(kc == KC - 1))
                        vs = ap.tile([128, DH], BF16, tag=f"v_s{kt}", bufs=2,
                                     name=f"v_s{kt}")
                        nc.vector.tensor_copy(out=vs[:], in_=v_p[:])
                        v_s.append(vs)
                    for qt in range(2):
                        s_p = aps.tile([128, n_keys], F32, space="PSUM", tag="s", bufs=1,
                                       name="s_p")
                        nc.tensor.matmul(out=s_p[:], lhsT=qt_s[:, qt * 128:(qt + 1) * 128],
                                         rhs=kt_s[:], start=True, stop=True)
                        rowsum = tp.tile([128, 1], F32, tag="rowsum", bufs=2,
                                         name="rowsum")
                        p_s = ap.tile([128, n_keys], BF16, tag="p_s", bufs=2, name="p_s")
                        if masks is not None:
                            sm = ap.tile([128, n_keys], F32, tag="sm", bufs=2, name="sm")
                            nc.vector.tensor_tensor(out=sm[:], in0=s_p[:],
                                                    in1=masks[qt][:], op=Alu.add)
                            nc.scalar.activation(out=p_s[:], in_=sm[:], func=Act.Exp,
                                                 scale=DH ** -0.5, accum_out=rowsum[:])
                        else:
                            nc.scalar.activation(out=p_s[:], in_=s_p[:], func=Act.Exp,
                                                 scale=DH ** -0.5, accum_out=rowsum[:])
                        rinv = tp.tile([128, 1], F32, tag="prinv", bufs=2, name="prinv")
                        nc.vector.reciprocal(out=rinv[:], in_=rowsum[:])
                        o_p = aps.tile([128, DH], F32, space="PSUM", tag="o", bufs=1,
                                       name="o_p")
                        for kt in range(nkt):
                            pt = aps.tile([128, 128], BF16, space="PSUM", tag="tr",
                                          bufs=2, name="ptp")
                            nc.tensor.transpose(out=pt[:],
                                                in_=p_s[:, kt * 128:(kt + 1) * 128],
                                                identity=identb[:])
                            pt_s = ap.tile([128, 128], BF16, tag="pt_s", bufs=2,
                                           name="pt_s")
                            nc.vector.tensor_copy(out=pt_s[:], in_=pt[:])
                            nc.tensor.matmul(out=o_p[:], lhsT=pt_s[:], rhs=v_s[kt][:],
                                             start=(kt == 0), stop=(kt == nkt - 1))
                        nc.vector.tensor_scalar_mul(out=attn[qt][:, hs], in0=o_p[:],
                                                    scalar1=rinv[:, :1])
                attnT = transpose_bf(attn, 2, f"{out_tag}_aT", ap)
                outs = []
                for qt in range(2):
                    pr = ps_misc.tile([128, D], F32, space="PSUM", tag="misc",
                                      name="proj")
                    for kc in range(KC):
                        nc.tensor.matmul(out=pr[:],
                                         lhsT=attnT[kc][:, qt * 128:(qt + 1) * 128],
                                         rhs=wo[kc][:], start=(kc == 0),
                                         stop=(kc == KC - 1))
                    xo = pers.tile([128, D], F32, tag=f"{out_tag}_x{qt}",
                                   name=f"{out_tag}_x{qt}")
                    nc.vector.tensor_tensor(out=xo[:], in0=resid[qt][:], in1=pr[:],
                                            op=Alu.add)
                    outs.append(xo)
                return outs

            # phase A: norm1 + self-attention
            with nc.named_scope("A_self"):
                n1 = rmsnorm(xb_t, wnb[0], "n1", 4, ap)
                n1T = transpose_bf(n1, 4, "n1T", ap)
                qT_self = [n1T[kc][:, 0:S] for kc in range(KC)]
                x1 = attention(qT_self, n1T, T, wt["sa_wq"], wt["sa_wk"], wt["sa_wv"],
                               wt["sa_wo"], mask_t, xb_t, "sa")

            # phase B: norm2 + cross-attention
            with nc.named_scope("B_cross"):
                n2 = rmsnorm(x1, wnb[1], "n2", 2, ap)
                n2T = transpose_bf(n2, 2, "n2T", ap)
                encT = transpose_bf(enc_bf, 4, "encT", ap)
                qT_cross = [n2T[kc][:, 0:S] for kc in range(KC)]
                x2_l = attention(qT_cross, encT, T, wt["ca_wq"], wt["ca_wk"],
                                 wt["ca_wv"], wt["ca_wo"], None, x1, "ca")
                x2[0], x2[1] = x2_l[0], x2_l[1]

            # phase C: norm3 + router logits + local top-2 + packed payload
            n3f = rmsnorm(x2, wnb[2], "n3f", 2, ap, out_dtype=F32)
            n3T = [ap.tile([128, S], F32, tag=f"n3T{kc}", name=f"n3T{kc}")
                   for kc in range(KC)]
            for i in range(2):
                for kc in range(KC):
                    pt = aps.tile([128, 128], F32, space="PSUM", tag="tr", bufs=2,
                                  name="n3tp")
                    nc.tensor.transpose(out=pt[:], in_=n3f[i][:, kc * 128:(kc + 1) * 128],
                                        identity=identf[:])
                    nc.vector.tensor_copy(out=n3T[kc][:, i * 128:(i + 1) * 128],
                                          in_=pt[:])
            lt_p = ps_misc.tile([E, S], F32, space="PSUM", tag="misc", name="lt_p")
            for kc in range(KC):
                nc.tensor.matmul(out=lt_p[:], lhsT=router_t[:, kc * E:(kc + 1) * E],
                                 rhs=n3T[kc][:], start=(kc == 0), stop=(kc == KC - 1))
            lt_s = tp.tile([E, S], F32, tag="lt_s", name="lt_s")
            nc.vector.tensor_copy(out=lt_s[:], in_=lt_p[:])

            # local top-2 per 128-token tile; pack (token_id + gate) per expert,
            # -1 for unselected.  pkE [E, 256] goes out for the tiny AG.
            pkE = ap.tile([E, 2 * 128], F32, tag="pkE", name="pkE")
            for i in range(2):
                pt = aps.tile([128, E], F32, space="PSUM", tag="tr", bufs=2,
                              name="ltT")
                nc.tensor.transpose(out=pt[:], in_=lt_s[:, i * 128:(i + 1) * 128],
                                    identity=identf[:E, :E])
                Lt = tp.tile([128, E], F32, tag="Lt", bufs=2, name="Lt")
                nc.vector.tensor_copy(out=Lt[:], in_=pt[:])
                m1 = tp.tile([128, 1], F32, tag="rm1", bufs=2, name="rm1")
                nc.vector.tensor_reduce(out=m1[:], in_=Lt[:], axis=Ax.X, op=Alu.max)
                mk1 = tp.tile([128, E], F32, tag="rmk1", bufs=2, name="rmk1")
                nc.vector.tensor_scalar(out=mk1[:], in0=Lt[:], scalar1=m1[:, :1],
                                        scalar2=None, op0=Alu.is_equal)
                L2 = tp.tile([128, E], F32, tag="rL2", bufs=2, name="rL2")
                nc.vector.scalar_tensor_tensor(out=L2[:], in0=mk1[:], scalar=-1e30,
                                               in1=Lt[:], op0=Alu.mult, op1=Alu.add)
                m2 = tp.tile([128, 1], F32, tag="rm2", bufs=2, name="rm2")
                nc.vector.tensor_reduce(out=m2[:], in_=L2[:], axis=Ax.X, op=Alu.max)
                mk2 = tp.tile([128, E], F32, tag="rmk2", bufs=2, name="rmk2")
                nc.vector.tensor_scalar(out=mk2[:], in0=L2[:], scalar1=m2[:, :1],
                                        scalar2=None, op0=Alu.is_equal)
                d_ = tp.tile([128, 1], F32, tag="rd", bufs=2, name="rd")
                nc.vector.tensor_tensor(out=d_[:], in0=m2[:], in1=m1[:],
                                        op=Alu.subtract)
                ed = tp.tile([128, 1], F32, tag="red", bufs=2, name="red")
                nc.scalar.activation(out=ed[:], in_=d_[:], func=Act.Exp)
                den = tp.tile([128, 1], F32, tag="rden", bufs=2, name="rden")
                nc.vector.tensor_scalar_add(out=den[:], in0=ed[:], scalar1=1.0)
                g1 = tp.tile([128, 1], F32, tag="rg1", bufs=2, name="rg1")
                nc.vector.reciprocal(out=g1[:], in_=den[:])
                # clamp top gate below 1.0 so floor(id+gate) stays id
                nc.vector.tensor_scalar_min(out=g1[:], in0=g1[:],
                                            scalar1=1.0 - 2.0 ** -12)
                g2 = tp.tile([128, 1], F32, tag="rg2", bufs=2, name="rg2")
                nc.vector.tensor_tensor(out=g2[:], in0=ed[:], in1=g1[:], op=Alu.mult)
                v1 = tp.tile([128, 1], F32, tag="rv1", bufs=2, name="rv1")
                nc.vector.tensor_tensor(out=v1[:], in0=gids_t[:, i:i + 1], in1=g1[:],
                                        op=Alu.add)
                v2 = tp.tile([128, 1], F32, tag="rv2", bufs=2, name="rv2")
                nc.vector.tensor_tensor(out=v2[:], in0=gids_t[:, i:i + 1], in1=g2[:],
                                        op=Alu.add)
                pk = tp.tile([128, E], F32, tag="rpk", bufs=2, name="rpk")
                nc.vector.tensor_scalar_mul(out=pk[:], in0=mk1[:], scalar1=v1[:, :1])
                nc.vector.scalar_tensor_tensor(out=pk[:], in0=mk2[:],
                                               scalar=v2[:, :1], in1=pk[:],
                                               op0=Alu.mult, op1=Alu.add)
                m12 = tp.tile([128, E], F32, tag="rm12", bufs=2, name="rm12")
                nc.vector.tensor_tensor(out=m12[:], in0=mk1[:], in1=mk2[:], op=Alu.add)
                nc.vector.tensor_scalar_add(out=m12[:], in0=m12[:], scalar1=-1.0)
                nc.vector.tensor_tensor(out=pk[:], in0=pk[:], in1=m12[:], op=Alu.add)
                ptb = aps.tile([E, 128], F32, space="PSUM", tag="tr", bufs=2,
                               name="pkT")
                nc.tensor.transpose(out=ptb[:], in_=pk[:], identity=identf[:])
                nc.vector.tensor_copy(out=pkE[:, i * 128:(i + 1) * 128], in_=ptb[:])
            nc.sync.dma_start(
                out=pk_sh[:].rearrange("(e h) f -> e (h f)", h=2), in_=pkE[:])

            # xn shard (bf16) for the big AG
            for i in range(2):
                nbf = pers.tile([128, D], BF16, tag=f"n3b{i}", name=f"n3b{i}")
                nc.vector.tensor_copy(out=nbf[:], in_=n3f[i][:])
                nc.sync.dma_start(out=xn_sh[i * 128:(i + 1) * 128, :], in_=nbf[:])

        # ================= allgathers: tiny routing payload first =========
        grp = [list(range(NCORES))]
        with nc.named_scope("AG"):
            nc.gpsimd.collective_compute("AllGather", Alu.bypass, replica_groups=grp,
                                         ins=[pk_sh[:].opt()], outs=[pk_all[:].opt()])
            nc.gpsimd.collective_compute("AllGather", Alu.bypass, replica_groups=grp,
                                         ins=[xn_sh[:].opt()], outs=[xn_all[:].opt()])

        # ================= phases D-F in a second scope =================
        with tc.tile_pool(name="moe", bufs=1) as mp, \
             tc.tile_pool(name="moe_ps", bufs=1, space="PSUM") as mps:

            # ---- routing: per-expert compaction via sparse_gather ----
            rt_ctx = nc.named_scope("Routing")
            rt_ctx.__enter__()
            ids_e = []    # [EPC][2] I32 [128, 1] token ids per slot chunk
            gates_e = []  # [EPC][2] F32 [128, 1] gate per slot chunk
            for e in range(EPC):
                # gather this expert's 16 payload rows from the AG'd pk
                sg_in = mp.tile([16, 128], F32, tag=f"sgin{e}", name=f"sgin{e}")
                nc.gpsimd.indirect_dma_start(
                    out=sg_in[:], out_offset=None, in_=pk_all[:],
                    in_offset=bass.IndirectOffsetOnAxis(
                        ap=eidx_t[:, e:e + 1], axis=0),
                    bounds_check=NCORES * E * 2 - 1, oob_is_err=False)
                sg_out = mp.tile([16, 16], F32, tag=f"sgout{e}", name=f"sgout{e}")
                nf = tp.tile([1, 1], U32, tag=f"nf{e}", name=f"nf{e}")
                nc.gpsimd.sparse_gather(out=sg_out[:], in_=sg_in[:],
                                        num_found=nf[:])
                nff = tp.tile([1, 1], F32, tag=f"nff{e}", name=f"nff{e}")
                nc.vector.tensor_copy(out=nff[:], in_=nf[:])
                nfp = ps_misc.tile([128, 1], F32, space="PSUM", tag="misc",
                                   name=f"nfp{e}")
                nc.tensor.matmul(out=nfp[:], lhsT=ones_f[:], rhs=nff[:],
                                 start=True, stop=True)
                nfb = tp.tile([128, 1], F32, tag=f"nfb{e}", name=f"nfb{e}")
                nc.vector.tensor_copy(out=nfb[:], in_=nfp[:])
                # transpose -> linear slot order, roundtrip to [128, 1] columns
                ptc = mps.tile([16, 16], F32, space="PSUM", tag="tr2", bufs=2,
                               name="sgTp")
                nc.tensor.transpose(out=ptc[:], in_=sg_out[:],
                                    identity=identf[:16, :16])
                sgT = mp.tile([16, 16], F32, tag=f"sgT{e}", name=f"sgT{e}")
                nc.vector.tensor_copy(out=sgT[:], in_=ptc[:])
                nc.sync.dma_start(
                    out=sg_scr[e][:].rearrange("a b -> (a b)")
                    .rearrange("(p f) -> p f", p=16), in_=sgT[:])
                ids_i, gates_i = [], []
                for ct in range(2):
                    clv = tp.tile([128, 1], F32, tag=f"clv{e}_{ct}",
                                  name=f"clv{e}_{ct}")
                    nc.sync.dma_start(
                        out=clv[:],
                        in_=sg_scr[e][ct:ct + 1, :]
                        .rearrange("one r -> (one r)")
                        .rearrange("(r one) -> r one", one=1))
                    keep = tp.tile([128, 1], F32, tag=f"kp{e}_{ct}",
                                   name=f"kp{e}_{ct}")
                    nc.vector.tensor_tensor(out=keep[:], in0=nfb[:],
                                            in1=sidx_t[:, ct:ct + 1], op=Alu.is_gt)
                    nc.vector.tensor_scalar_add(out=clv[:], in0=clv[:], scalar1=1.0)
                    nc.vector.tensor_tensor(out=clv[:], in0=clv[:], in1=keep[:],
                                            op=Alu.mult)
                    nc.vector.tensor_scalar_add(out=clv[:], in0=clv[:], scalar1=-1.0)
                    # unpack id + gate (empty slots are -1 -> id 0, gate 0).
                    # floor via int-cast roundtrip; correct in case the cast
                    # rounds up instead of truncating.
                    nc.vector.tensor_scalar_max(out=clv[:], in0=clv[:], scalar1=0.0)
                    ids0 = tp.tile([128, 1], I32, tag=f"ids0{e}_{ct}",
                                   name=f"ids0{e}_{ct}")
                    nc.vector.tensor_copy(out=ids0[:], in_=clv[:])
                    idf = tp.tile([128, 1], F32, tag=f"idf{e}_{ct}",
                                  name=f"idf{e}_{ct}")
                    nc.vector.tensor_copy(out=idf[:], in_=ids0[:])
                    wrong = tp.tile([128, 1], F32, tag=f"wr{e}_{ct}",
                                    name=f"wr{e}_{ct}")
                    nc.vector.tensor_tensor(out=wrong[:], in0=idf[:], in1=clv[:],
                                            op=Alu.is_gt)
                    nc.vector.tensor_tensor(out=idf[:], in0=idf[:], in1=wrong[:],
                                            op=Alu.subtract)
                    gat = tp.tile([128, 1], F32, tag=f"gat{e}_{ct}",
                                  name=f"gat{e}_{ct}")
                    nc.vector.tensor_tensor(out=gat[:], in0=clv[:], in1=idf[:],
                                            op=Alu.subtract)
                    ids = tp.tile([128, 1], I32, tag=f"ids{e}_{ct}",
                                  name=f"ids{e}_{ct}")
                    nc.vector.tensor_copy(out=ids[:], in_=idf[:])
                    ids_i.append(ids)
                    gates_i.append(gat)
                ids_e.append(ids_i)
                gates_e.append(gates_i)
            rt_ctx.__exit__(None, None, None)

            # ---- expert compute ----
            for e in range(EPC):
                e_ctx = nc.named_scope(f"Expert{e}")
                e_ctx.__enter__()
                ids_i = ids_e[e]
                gates_i = gates_e[e]
                xeT = [mp.tile([128, CAP], BF16, tag=f"xeT_{kc}", bufs=2,
                               name=f"xeT{e}_{kc}") for kc in range(KC)]
                for ct in range(2):
                    xe = mp.tile([128, D], BF16, tag="xe", bufs=2, name=f"xe{e}_{ct}")
                    nc.gpsimd.indirect_dma_start(
                        out=xe[:], out_offset=None, in_=xn_all[:],
                        in_offset=bass.IndirectOffsetOnAxis(
                            ap=ids_i[ct][:, :1], axis=0),
                        bounds_check=NTOK - 1, oob_is_err=False)
                    for kc in range(KC):
                        pt = mps.tile([128, 128], BF16, space="PSUM", tag="tr2",
                                      bufs=2, name="xetp")
                        nc.tensor.transpose(out=pt[:],
                                            in_=xe[:, kc * 128:(kc + 1) * 128],
                                            identity=identb[:])
                        nc.vector.tensor_copy(out=xeT[kc][:, ct * 128:(ct + 1) * 128],
                                              in_=pt[:])
                wd_e = []
                for hc in range(HC):
                    t_ = wdp.tile([128, D], BF16, tag="wd", name=f"wd{e}_{hc}")
                    nc.gpsimd.dma_start(out=t_[:],
                                        in_=wd_in[e, hc * 128:(hc + 1) * 128, :])
                    wd_e.append(t_)
                aT = []
                for q in range(QH):
                    wg_q, wu_q = [], []
                    for kc in range(KC):
                        tg = moew.tile([128, 512], BF16, tag="wgq",
                                       name=f"wg{e}_{q}_{kc}")
                        nc.gpsimd.dma_start(
                            out=tg[:], in_=wg_in[e, kc * 128:(kc + 1) * 128,
                                                 q * 512:(q + 1) * 512])
                        wg_q.append(tg)
                        tu = moew.tile([128, 512], BF16, tag="wuq",
                                       name=f"wu{e}_{q}_{kc}")
                        nc.gpsimd.dma_start(
                            out=tu[:], in_=wu_in[e, kc * 128:(kc + 1) * 128,
                                                 q * 512:(q + 1) * 512])
                        wu_q.append(tu)
                    for hcl in range(4):
                        hsl = slice(hcl * 128, (hcl + 1) * 128)
                        hT = mps.tile([128, CAP], F32, space="PSUM", tag="hu", bufs=3,
                                      name="hT")
                        for kc in range(KC):
                            nc.tensor.matmul(out=hT[:], lhsT=wg_q[kc][:, hsl],
                                             rhs=xeT[kc][:],
                                             start=(kc == 0), stop=(kc == KC - 1))
                        uT = mps.tile([128, CAP], F32, space="PSUM", tag="hu", bufs=3,
                                      name="uT")
                        for kc in range(KC):
                            nc.tensor.matmul(out=uT[:], lhsT=wu_q[kc][:, hsl],
                                             rhs=xeT[kc][:],
                                             start=(kc == 0), stop=(kc == KC - 1))
                        sl = tp.tile([128, CAP], BF16, tag="silu", bufs=2, name="silu")
                        if SIM_SAFE_SILU:
                            sgm = tp.tile([128, CAP], F32, tag="sgm", bufs=2,
                                          name="sgm")
                            nc.scalar.activation(out=sgm[:], in_=hT[:],
                                                 func=Act.Sigmoid)
                            nc.vector.tensor_tensor(out=sl[:], in0=sgm[:],
                                                    in1=hT[:], op=Alu.mult)
                        else:
                            nc.scalar.activation(out=sl[:], in_=hT[:], func=Act.Silu)
                        a_ = mp.tile([128, CAP], BF16, tag="aT", bufs=20,
                                     name=f"aT{e}_{q}_{hcl}")
                        nc.vector.tensor_tensor(out=a_[:], in0=sl[:], in1=uT[:],
                                                op=Alu.mult)
                        aT.append(a_)
                for ct in range(2):
                    y_p = mps.tile([128, D], F32, space="PSUM", tag="y", bufs=1,
                                   name="y_p")
                    for hc in range(HC):
                        nc.tensor.matmul(out=y_p[:],
                                         lhsT=aT[hc][:, ct * 128:(ct + 1) * 128],
                                         rhs=wd_e[hc][:], start=(hc == 0),
                                         stop=(hc == HC - 1))
                    y_g = tp.tile([128, D], BF16, tag="y_g", bufs=2, name=f"y_g{e}_{ct}")
                    nc.vector.tensor_scalar_mul(out=y_g[:], in0=y_p[:],
                                                scalar1=gates_i[ct][:, :1])
                    nc.gpsimd.indirect_dma_start(
                        out=accum[:], out_offset=bass.IndirectOffsetOnAxis(
                            ap=ids_i[ct][:, :1], axis=0),
                        in_=y_g[:], in_offset=None, compute_op=Alu.add,
                        bounds_check=NTOK - 1, oob_is_err=False)
                e_ctx.__exit__(None, None, None)

            # ---- reduce-scatter + residual ----
            with nc.named_scope("RS"):
                nc.gpsimd.collective_compute("ReduceScatter", Alu.add,
                                             replica_groups=grp,
                                             ins=[accum[:].opt()],
                                             outs=[rs_out[:].opt()])
            for qt in range(2):
                rs_t = tp.tile([128, D], BF16, tag=f"rs{qt}", name=f"rs{qt}")
                nc.sync.dma_start(out=rs_t[:], in_=rs_out[qt * 128:(qt + 1) * 128, :])
                o_t = tp.tile([128, D], F32, tag=f"ofin{qt}", name=f"ofin{qt}")
                nc.vector.tensor_tensor(out=o_t[:], in0=x2[qt][:], in1=rs_t[:],
                                        op=Alu.add)
                nc.sync.dma_start(out=out_dram[qt * 128:(qt + 1) * 128, :], in_=o_t[:])

    nc.compile()
    return nc


_NC_CACHE = None


def _get_program():
    global _NC_CACHE
    if _NC_CACHE is None:
        _NC_CACHE = build_program()
    return _NC_CACHE


def make_in_maps(x, enc_out, causal_mask, norm1_w, norm2_w, norm3_w,
                 sa_wq, sa_wk, sa_wv, sa_wo, ca_wq, ca_wk, ca_wv, ca_wo,
                 router_w, moe_wg, moe_wu, moe_wd):
    x = np.asarray(x, np.float32)
    enc_out = np.asarray(enc_out, np.float32)
    causal_mask = np.asarray(causal_mask)
    fullmask = np.where(causal_mask, np.float32(-1e30), np.float32(0.0))
    norms = np.stack([np.asarray(norm1_w, np.float32),
                      np.asarray(norm2_w, np.float32),
                      np.asarray(norm3_w, np.float32)], 0)
    shared = {
        "norms": norms,
        "router_w": np.asarray(router_w, np.float32),
        "sa_wq": np.asarray(sa_wq, np.float32), "sa_wk": np.asarray(sa_wk, np.float32),
        "sa_wv": np.asarray(sa_wv, np.float32), "sa_wo": np.asarray(sa_wo, np.float32),
        "ca_wq": np.asarray(ca_wq, np.float32), "ca_wk": np.asarray(ca_wk, np.float32),
        "ca_wv": np.asarray(ca_wv, np.float32), "ca_wo": np.asarray(ca_wo, np.float32),
    }
    moe_wg = np.asarray(moe_wg, np.float32).astype(ml_dtypes.bfloat16)
    moe_wu = np.asarray(moe_wu, np.float32).astype(ml_dtypes.bfloat16)
    moe_wd = np.asarray(moe_wd, np.float32).astype(ml_dtypes.bfloat16)

    in_maps = []
    for c in range(NCORES):
        b, h = c // 2, c % 2
        perm = np.concatenate([np.arange(h * S, (h + 1) * S),
                               np.arange((1 - h) * S, (2 - h) * S)])
        xb_perm = x[b][perm]
        mrows = fullmask[h * S:(h + 1) * S][:, perm]
        # rows of pk_all [NCORES*E*2, 128] holding this core's 3 experts
        eidx = np.empty((16, EPC), np.int32)
        for i in range(EPC):
            eg = EPC * c + i
            for p in range(16):
                eidx[p, i] = (p // 2) * E * 2 + eg * 2 + (p % 2)
        # global token ids for this core's two 128-token tiles
        gids = (np.float32(c * S)
                + np.arange(128, dtype=np.float32)[:, None]
                + np.float32(128) * np.arange(2, dtype=np.float32)[None, :])
        m = dict(shared)
        m["xb"] = np.ascontiguousarray(xb_perm)
        m["encb"] = np.ascontiguousarray(enc_out[b])
        m["maskadd"] = np.ascontiguousarray(mrows)
        m["eidx"] = eidx
        m["gids"] = np.ascontiguousarray(gids)
        m["sidx"] = np.ascontiguousarray(
            np.arange(128, dtype=np.float32)[:, None]
            + np.float32(128) * np.arange(2, dtype=np.float32)[None, :])
        m["wg"] = np.ascontiguousarray(moe_wg[EPC * c:EPC * (c + 1)])
        m["wu"] = np.ascontiguousarray(moe_wu[EPC * c:EPC * (c + 1)])
        m["wd"] = np.ascontiguousarray(moe_wd[EPC * c:EPC * (c + 1)])
        in_maps.append(m)
    return in_maps


def assemble_out(results):
    out = np.empty((B, T, D), np.float32)
    for c in range(NCORES):
        b, h = c // 2, c % 2
        out[b, h * S:(h + 1) * S] = results[c]["out"]
    return out


def kernel(**inputs):
    nc = _get_program()
    in_maps = make_in_maps(**inputs)
    res = run_bass_kernel_spmd(nc, in_maps, list(range(NCORES)))
    return assemble_out(res.results)


if __name__ == "__main__":
    import reference
    inp = reference.setup_inputs()
    got = kernel(**{k: np.asarray(v) for k, v in inp.items()})
    exp = np.asarray(reference.reference(**inp))
    err = np.abs(got - exp)
    print("abs max err:", err.max(), "rel:", err.max() / np.abs(exp).max())



# revision 32
# speedup vs baseline: 1.3619x; 1.0221x over previous
"""Trainium2 Bass kernel for nn_DecoderLayer_65266323030558.

Decoder layer: rmsnorm -> causal self-attn -> rmsnorm -> cross-attn ->
rmsnorm -> top-2-of-24 MoE (sparse: compute only the routed experts).

Sharding (8 cores):
  - Attention: token-parallel. Core c handles batch c//2, T-half c%2.
    Host permutes each core's batch rows so its 256 query tokens are
    always rows 0:256 (uniform SPMD program); the causal mask columns
    are permuted to match and shipped as an additive f32 mask.
  - MoE: expert-parallel (3 experts/core). Normed tokens are AllGathered
    (bf16) along with fp32 router logits; each core compacts the token
    lists for its 3 experts on-device (top-2 + prefix-scan + indirect
    DMA), computes only routed tokens, scatter-adds gated outputs into a
    bf16 accumulator, and a ReduceScatter returns each core its shard.

Matmuls run in bf16 (fp32 PSUM accumulation); norms, softmax, routing,
and residuals stay fp32.
"""
from contextlib import ExitStack

import numpy as np
import ml_dtypes

import concourse.bass as bass
import concourse.mybir as mybir
import concourse.tile as tile
from concourse import bacc, library_config
from concourse.bass_utils import run_bass_kernel_spmd
from concourse.masks import make_identity

F32 = mybir.dt.float32
BF16 = mybir.dt.bfloat16
I32 = mybir.dt.int32
U32 = mybir.dt.uint32
Alu = mybir.AluOpType
Act = mybir.ActivationFunctionType
Ax = mybir.AxisListType

SIM_SAFE_SILU = False  # sim_test sets True: CoreSim lacks Silu

B, T, D, H, DH, E, TOPK, HID = 4, 512, 512, 8, 64, 24, 2, 2048
EPS = 1e-6
NCORES = 8
S = 256            # tokens per core
NTOK = B * T       # 2048
CAP = 256          # per-expert token capacity
EPC = E // NCORES  # experts per core = 3
NT = NTOK // 128   # 16 token tiles
KC = D // 128      # 4 contraction chunks over D
HC = HID // 128    # 16 chunks over HID
QH = HID // 512    # 4 quarter-chunks of HID (streaming unit for wg/wu)


def build_program():
    nc = bacc.Bacc(num_devices=NCORES)

    # ---------------- I/O ----------------
    xb = nc.declare_dram_parameter("xb", [T, D], F32, isOutput=False)
    encb = nc.declare_dram_parameter("encb", [T, D], BF16, isOutput=False)
    maskadd = nc.declare_dram_parameter("maskadd", [S, T], F32, isOutput=False)
    eidx_in = nc.declare_dram_parameter("eidx", [16, EPC], I32, isOutput=False)
    gids_in = nc.declare_dram_parameter("gids", [128, 2], F32, isOutput=False)
    sidx_in = nc.declare_dram_parameter("sidx", [128, 2], F32, isOutput=False)
    norms_in = nc.declare_dram_parameter("norms", [3, D], F32, isOutput=False)
    router_in = nc.declare_dram_parameter("router_w", [D, E], F32, isOutput=False)
    wattn = {}
    for name in ["sa_wq", "sa_wk", "sa_wv", "sa_wo", "ca_wq", "ca_wk", "ca_wv", "ca_wo"]:
        wattn[name] = nc.declare_dram_parameter(name, [D, D], BF16, isOutput=False)
    wg_in = nc.declare_dram_parameter("wg", [EPC, D, HID], BF16, isOutput=False)
    wu_in = nc.declare_dram_parameter("wu", [EPC, D, HID], BF16, isOutput=False)
    wd_in = nc.declare_dram_parameter("wd", [EPC, HID, D], BF16, isOutput=False)
    out_dram = nc.declare_dram_parameter("out", [S, D], F32, isOutput=True)

    # internal DRAM
    xn_sh = nc.dram_tensor("xn_sh", [S, D], BF16)
    pk_sh = nc.dram_tensor("pk_sh", [E * 2, 128], F32)
    xn_all = nc.dram_tensor("xn_all", [NTOK, D], BF16, addr_space="Shared")
    pk_all = nc.dram_tensor("pk_all", [NCORES * E * 2, 128], F32,
                            addr_space="Shared")
    accum = nc.dram_tensor("accum", [NTOK, D], BF16)
    rs_out = nc.dram_tensor("rs_out", [S, D], BF16)
    sg_scr = [nc.dram_tensor(f"sg_scr{e}", [2, 128], F32) for e in range(EPC)]

    with tile.TileContext(nc) as tc, ExitStack() as ctx:
        # pools opened first get the low SBUF region and are never blocked
        # by later scoped-pool releases.
        const = ctx.enter_context(tc.tile_pool(name="const", bufs=1))
        moew = ctx.enter_context(tc.tile_pool(name="moew", bufs=12))
        wdp = ctx.enter_context(tc.tile_pool(name="wdp", bufs=16))
        wpool = ctx.enter_context(tc.tile_pool(name="wpool", bufs=1))
        pers = ctx.enter_context(tc.tile_pool(name="pers", bufs=1))
        tp = ctx.enter_context(tc.tile_pool(name="tp", bufs=1))
        ps_misc = ctx.enter_context(tc.tile_pool(name="ps_misc", bufs=1, space="PSUM"))

        # ------------- constants -------------
        identb = const.tile([128, 128], BF16)
        make_identity(nc, identb[:])
        identf = const.tile([128, 128], F32)
        make_identity(nc, identf[:])
        ones_f = const.tile([1, 128], F32)
        nc.vector.memset(ones_f[:], 1.0)
        eps_t = const.tile([128, 1], F32)
        nc.vector.memset(eps_t[:], EPS)
        wnb = []
        for i in range(3):
            nrow = const.tile([1, D], F32, tag=f"nrow{i}", name=f"nrow{i}")
            nc.sync.dma_start(out=nrow[:], in_=norms_in[i:i + 1, :])
            pb = ps_misc.tile([128, D], F32, space="PSUM", tag="misc", name=f"wnbp{i}")
            nc.tensor.matmul(out=pb[:], lhsT=ones_f[:], rhs=nrow[:], start=True, stop=True)
            wb = const.tile([128, D], F32, tag=f"wnb{i}", name=f"wnb{i}")
            nc.vector.tensor_copy(out=wb[:], in_=pb[:])
            wnb.append(wb)
        eidx_t = const.tile([16, EPC], I32)
        nc.sync.dma_start(out=eidx_t[:], in_=eidx_in[:])
        gids_t = const.tile([128, 2], F32)
        nc.sync.dma_start(out=gids_t[:], in_=gids_in[:])
        sidx_t = const.tile([128, 2], F32)
        nc.sync.dma_start(out=sidx_t[:], in_=sidx_in[:])
        router_t = const.tile([128, KC * E], F32)
        for kc in range(KC):
            nc.sync.dma_start(out=router_t[:, kc * E:(kc + 1) * E],
                              in_=router_in[kc * 128:(kc + 1) * 128, :])

        # zero-init dram targets early
        zbf = const.tile([128, 512], BF16)
        nc.vector.memset(zbf[:], 0.0)
        for i in range(NT):
            nc.sync.dma_start(out=accum[i * 128:(i + 1) * 128, :], in_=zbf[:])

        # ------------- attention weight loads (cast f32->bf16 in DMA) ----
        wt = {}
        for name in wattn:
            tiles = []
            for kc in range(KC):
                t_ = wpool.tile([128, D], BF16, tag=f"{name}_{kc}", name=f"{name}_{kc}")
                nc.gpsimd.dma_start(out=t_[:], in_=wattn[name][kc * 128:(kc + 1) * 128, :])
                tiles.append(t_)
            wt[name] = tiles

        # ------------- residual x tiles (f32) -------
        xb_t = []
        for i in range(4):
            t_ = pers.tile([128, D], F32, tag=f"xb{i}", name=f"xb{i}")
            nc.sync.dma_start(out=t_[:], in_=xb[i * 128:(i + 1) * 128, :])
            xb_t.append(t_)

        x2 = [None, None]  # filled inside the attention scope

        # ================= phases A-C in a scoped pool =================
        with tc.tile_pool(name="attn", bufs=1) as ap, \
             tc.tile_pool(name="attn_ps", bufs=1, space="PSUM") as aps:

            enc_bf = []
            for i in range(4):
                t_ = ap.tile([128, D], BF16, tag=f"enc{i}", name=f"enc{i}")
                nc.gpsimd.dma_start(out=t_[:], in_=encb[i * 128:(i + 1) * 128, :])
                enc_bf.append(t_)
            mask_t = []
            for i in range(2):
                t_ = ap.tile([128, T], F32, tag=f"mask{i}", name=f"mask{i}")
                nc.sync.dma_start(out=t_[:], in_=maskadd[i * 128:(i + 1) * 128, :])
                mask_t.append(t_)

            def rmsnorm(x_tiles, w_bcast, out_tag, n_tiles, pool, out_dtype=BF16):
                outs = []
                for i in range(n_tiles):
                    sq = ps_misc.tile([128, D], F32, space="PSUM", tag="misc",
                                      name=f"sq_{out_tag}{i}")
                    acc = tp.tile([128, 1], F32, tag="sqacc", bufs=2, name="sqacc")
                    nc.scalar.activation(out=sq[:], in_=x_tiles[i][:], func=Act.Square,
                                         accum_out=acc[:])
                    rms = tp.tile([128, 1], F32, tag="rms", bufs=2, name="rms")
                    nc.scalar.activation(out=rms[:], in_=acc[:], func=Act.Sqrt,
                                         scale=1.0 / D, bias=eps_t[:, :1])
                    rinv = tp.tile([128, 1], F32, tag="rinv", bufs=2, name="rinv")
                    nc.vector.reciprocal(out=rinv[:], in_=rms[:])
                    nt_ = pool.tile([128, D], out_dtype, tag=f"{out_tag}{i}",
                                    name=f"{out_tag}{i}")
                    nc.vector.scalar_tensor_tensor(out=nt_[:], in0=x_tiles[i][:],
                                                   scalar=rinv[:, :1], in1=w_bcast[:],
                                                   op0=Alu.mult, op1=Alu.mult)
                    outs.append(nt_)
                return outs

            def transpose_bf(src_tiles, n_src, out_tag, pool):
                outs = [pool.tile([128, 128 * n_src], BF16, tag=f"{out_tag}{kc}",
                                  name=f"{out_tag}{kc}") for kc in range(KC)]
                for i in range(n_src):
                    for kc in range(KC):
                        pt = aps.tile([128, 128], BF16, space="PSUM", tag="tr",
                                      bufs=2, name="trp")
                        nc.tensor.transpose(out=pt[:],
                                            in_=src_tiles[i][:, kc * 128:(kc + 1) * 128],
                                            identity=identb[:])
                        nc.vector.tensor_copy(out=outs[kc][:, i * 128:(i + 1) * 128],
                                              in_=pt[:])
                return outs

            def attention(qT, kvT, n_keys, wq, wk, wv, wo, masks, resid, out_tag):
                nkt = n_keys // 128
                attn = [ap.tile([128, D], BF16, tag=f"{out_tag}_a{qt}",
                                name=f"{out_tag}_a{qt}") for qt in range(2)]
                for h in range(H):
                    hs = slice(h * DH, (h + 1) * DH)
                    kt_p = aps.tile([DH, n_keys], F32, space="PSUM", tag="kqv", bufs=2,
                                    name="kt_p")
                    for kc in range(KC):
                        nc.tensor.matmul(out=kt_p[:], lhsT=wk[kc][:, hs], rhs=kvT[kc][:],
                                         start=(kc == 0), stop=(kc == KC - 1))
                    kt_s = ap.tile([DH, n_keys], BF16, tag="kt_s", bufs=2, name="kt_s")
                    nc.vector.tensor_copy(out=kt_s[:], in_=kt_p[:])
                    qt_p = aps.tile([DH, S], F32, space="PSUM", tag="kqv", bufs=2,
                                    name="qt_p")
                    for kc in range(KC):
                        nc.tensor.matmul(out=qt_p[:], lhsT=wq[kc][:, hs], rhs=qT[kc],
                                         start=(kc == 0), stop=(kc == KC - 1))
                    qt_s = ap.tile([DH, S], BF16, tag="qt_s", bufs=2, name="qt_s")
                    nc.vector.tensor_copy(out=qt_s[:], in_=qt_p[:])
                    v_s = []
                    for kt in range(nkt):
                        v_p = aps.tile([128, DH], F32, space="PSUM", tag="kqv", bufs=2,
                                       name="v_p")
                        for kc in range(KC):
                            nc.tensor.matmul(out=v_p[:],
                                             lhsT=kvT[kc][:, kt * 128:(kt + 1) * 128],
                                             rhs=wv[kc][:, hs],
                                             start=(kc == 0), stop=(kc == KC - 1))
                        vs = ap.tile([128, DH], BF16, tag=f"v_s{kt}", bufs=2,
                                     name=f"v_s{kt}")
                        nc.vector.tensor_copy(out=vs[:], in_=v_p[:])
                        v_s.append(vs)
                    for qt in range(2):
                        s_p = aps.tile([128, n_keys], F32, space="PSUM", tag="s", bufs=1,
                                       name="s_p")
                        nc.tensor.matmul(out=s_p[:], lhsT=qt_s[:, qt * 128:(qt + 1) * 128],
                                         rhs=kt_s[:], start=True, stop=True)
                        rowsum = tp.tile([128, 1], F32, tag="rowsum", bufs=2,
                                         name="rowsum")
                        p_s = ap.tile([128, n_keys], BF16, tag="p_s", bufs=2, name="p_s")
                        if masks is not None:
                            sm = ap.tile([128, n_keys], F32, tag="sm", bufs=2, name="sm")
                            nc.vector.tensor_tensor(out=sm[:], in0=s_p[:],
                                                    in1=masks[qt][:], op=Alu.add)
                            nc.scalar.activation(out=p_s[:], in_=sm[:], func=Act.Exp,
                                                 scale=DH ** -0.5, accum_out=rowsum[:])
                        else:
                            nc.scalar.activation(out=p_s[:], in_=s_p[:], func=Act.Exp,
                                                 scale=DH ** -0.5, accum_out=rowsum[:])
                        rinv = tp.tile([128, 1], F32, tag="prinv", bufs=2, name="prinv")
                        nc.vector.reciprocal(out=rinv[:], in_=rowsum[:])
                        o_p = aps.tile([128, DH], F32, space="PSUM", tag="o", bufs=1,
                                       name="o_p")
                        for kt in range(nkt):
                            pt = aps.tile([128, 128], BF16, space="PSUM", tag="tr",
                                          bufs=2, name="ptp")
                            nc.tensor.transpose(out=pt[:],
                                                in_=p_s[:, kt * 128:(kt + 1) * 128],
                                                identity=identb[:])
                            pt_s = ap.tile([128, 128], BF16, tag="pt_s", bufs=2,
                                           name="pt_s")
                            nc.vector.tensor_copy(out=pt_s[:], in_=pt[:])
                            nc.tensor.matmul(out=o_p[:], lhsT=pt_s[:], rhs=v_s[kt][:],
                                             start=(kt == 0), stop=(kt == nkt - 1))
                        nc.vector.tensor_scalar_mul(out=attn[qt][:, hs], in0=o_p[:],
                                                    scalar1=rinv[:, :1])
                attnT = transpose_bf(attn, 2, f"{out_tag}_aT", ap)
                outs = []
                for qt in range(2):
                    pr = ps_misc.tile([128, D], F32, space="PSUM", tag="misc",
                                      name="proj")
                    for kc in range(KC):
                        nc.tensor.matmul(out=pr[:],
                                         lhsT=attnT[kc][:, qt * 128:(qt + 1) * 128],
                                         rhs=wo[kc][:], start=(kc == 0),
                                         stop=(kc == KC - 1))
                    xo = pers.tile([128, D], F32, tag=f"{out_tag}_x{qt}",
                                   name=f"{out_tag}_x{qt}")
                    nc.vector.tensor_tensor(out=xo[:], in0=resid[qt][:], in1=pr[:],
                                            op=Alu.add)
                    outs.append(xo)
                return outs

            # phase A: norm1 + self-attention
            with nc.named_scope("A_self"):
                n1 = rmsnorm(xb_t, wnb[0], "n1", 4, ap)
                n1T = transpose_bf(n1, 4, "n1T", ap)
                qT_self = [n1T[kc][:, 0:S] for kc in range(KC)]
                x1 = attention(qT_self, n1T, T, wt["sa_wq"], wt["sa_wk"], wt["sa_wv"],
                               wt["sa_wo"], mask_t, xb_t, "sa")

            # phase B: norm2 + cross-attention
            with nc.named_scope("B_cross"):
                n2 = rmsnorm(x1, wnb[1], "n2", 2, ap)
                n2T = transpose_bf(n2, 2, "n2T", ap)
                encT = transpose_bf(enc_bf, 4, "encT", ap)
                qT_cross = [n2T[kc][:, 0:S] for kc in range(KC)]
                x2_l = attention(qT_cross, encT, T, wt["ca_wq"], wt["ca_wk"],
                                 wt["ca_wv"], wt["ca_wo"], None, x1, "ca")
                x2[0], x2[1] = x2_l[0], x2_l[1]

            # phase C: norm3 + router logits + local top-2 + packed payload
            n3f = rmsnorm(x2, wnb[2], "n3f", 2, ap, out_dtype=F32)
            n3T = [ap.tile([128, S], F32, tag=f"n3T{kc}", name=f"n3T{kc}")
                   for kc in range(KC)]
            for i in range(2):
                for kc in range(KC):
                    pt = aps.tile([128, 128], F32, space="PSUM", tag="tr", bufs=2,
                                  name="n3tp")
                    nc.tensor.transpose(out=pt[:], in_=n3f[i][:, kc * 128:(kc + 1) * 128],
                                        identity=identf[:])
                    nc.vector.tensor_copy(out=n3T[kc][:, i * 128:(i + 1) * 128],
                                          in_=pt[:])
            lt_p = ps_misc.tile([E, S], F32, space="PSUM", tag="misc", name="lt_p")
            for kc in range(KC):
                nc.tensor.matmul(out=lt_p[:], lhsT=router_t[:, kc * E:(kc + 1) * E],
                                 rhs=n3T[kc][:], start=(kc == 0), stop=(kc == KC - 1))
            lt_s = tp.tile([E, S], F32, tag="lt_s", name="lt_s")
            nc.vector.tensor_copy(out=lt_s[:], in_=lt_p[:])

            # local top-2 per 128-token tile; pack (token_id + gate) per expert,
            # -1 for unselected.  pkE [E, 256] goes out for the tiny AG.
            pkE = ap.tile([E, 2 * 128], F32, tag="pkE", name="pkE")
            for i in range(2):
                pt = aps.tile([128, E], F32, space="PSUM", tag="tr", bufs=2,
                              name="ltT")
                nc.tensor.transpose(out=pt[:], in_=lt_s[:, i * 128:(i + 1) * 128],
                                    identity=identf[:E, :E])
                Lt = tp.tile([128, E], F32, tag="Lt", bufs=2, name="Lt")
                nc.vector.tensor_copy(out=Lt[:], in_=pt[:])
                m1 = tp.tile([128, 1], F32, tag="rm1", bufs=2, name="rm1")
                nc.vector.tensor_reduce(out=m1[:], in_=Lt[:], axis=Ax.X, op=Alu.max)
                mk1 = tp.tile([128, E], F32, tag="rmk1", bufs=2, name="rmk1")
                nc.vector.tensor_scalar(out=mk1[:], in0=Lt[:], scalar1=m1[:, :1],
                                        scalar2=None, op0=Alu.is_equal)
                L2 = tp.tile([128, E], F32, tag="rL2", bufs=2, name="rL2")
                nc.vector.scalar_tensor_tensor(out=L2[:], in0=mk1[:], scalar=-1e30,
                                               in1=Lt[:], op0=Alu.mult, op1=Alu.add)
                m2 = tp.tile([128, 1], F32, tag="rm2", bufs=2, name="rm2")
                nc.vector.tensor_reduce(out=m2[:], in_=L2[:], axis=Ax.X, op=Alu.max)
                mk2 = tp.tile([128, E], F32, tag="rmk2", bufs=2, name="rmk2")
                nc.vector.tensor_scalar(out=mk2[:], in0=L2[:], scalar1=m2[:, :1],
                                        scalar2=None, op0=Alu.is_equal)
                d_ = tp.tile([128, 1], F32, tag="rd", bufs=2, name="rd")
                nc.vector.tensor_tensor(out=d_[:], in0=m2[:], in1=m1[:],
                                        op=Alu.subtract)
                ed = tp.tile([128, 1], F32, tag="red", bufs=2, name="red")
                nc.scalar.activation(out=ed[:], in_=d_[:], func=Act.Exp)
                den = tp.tile([128, 1], F32, tag="rden", bufs=2, name="rden")
                nc.vector.tensor_scalar_add(out=den[:], in0=ed[:], scalar1=1.0)
                g1 = tp.tile([128, 1], F32, tag="rg1", bufs=2, name="rg1")
                nc.vector.reciprocal(out=g1[:], in_=den[:])
                # clamp top gate below 1.0 so floor(id+gate) stays id
                nc.vector.tensor_scalar_min(out=g1[:], in0=g1[:],
                                            scalar1=1.0 - 2.0 ** -12)
                g2 = tp.tile([128, 1], F32, tag="rg2", bufs=2, name="rg2")
                nc.vector.tensor_tensor(out=g2[:], in0=ed[:], in1=g1[:], op=Alu.mult)
                v1 = tp.tile([128, 1], F32, tag="rv1", bufs=2, name="rv1")
                nc.vector.tensor_tensor(out=v1[:], in0=gids_t[:, i:i + 1], in1=g1[:],
                                        op=Alu.add)
                v2 = tp.tile([128, 1], F32, tag="rv2", bufs=2, name="rv2")
                nc.vector.tensor_tensor(out=v2[:], in0=gids_t[:, i:i + 1], in1=g2[:],
                                        op=Alu.add)
                pk = tp.tile([128, E], F32, tag="rpk", bufs=2, name="rpk")
                nc.vector.tensor_scalar_mul(out=pk[:], in0=mk1[:], scalar1=v1[:, :1])
                nc.vector.scalar_tensor_tensor(out=pk[:], in0=mk2[:],
                                               scalar=v2[:, :1], in1=pk[:],
                                               op0=Alu.mult, op1=Alu.add)
                m12 = tp.tile([128, E], F32, tag="rm12", bufs=2, name="rm12")
                nc.vector.tensor_tensor(out=m12[:], in0=mk1[:], in1=mk2[:], op=Alu.add)
                nc.vector.tensor_scalar_add(out=m12[:], in0=m12[:], scalar1=-1.0)
                nc.vector.tensor_tensor(out=pk[:], in0=pk[:], in1=m12[:], op=Alu.add)
                ptb = aps.tile([E, 128], F32, space="PSUM", tag="tr", bufs=2,
                               name="pkT")
                nc.tensor.transpose(out=ptb[:], in_=pk[:], identity=identf[:])
                nc.vector.tensor_copy(out=pkE[:, i * 128:(i + 1) * 128], in_=ptb[:])
            nc.sync.dma_start(
                out=pk_sh[:].rearrange("(e h) f -> e (h f)", h=2), in_=pkE[:])

            # xn shard (bf16) for the big AG
            for i in range(2):
                nbf = pers.tile([128, D], BF16, tag=f"n3b{i}", name=f"n3b{i}")
                nc.vector.tensor_copy(out=nbf[:], in_=n3f[i][:])
                nc.sync.dma_start(out=xn_sh[i * 128:(i + 1) * 128, :], in_=nbf[:])

        # ================= allgathers: tiny routing payload first =========
        grp = [list(range(NCORES))]
        with nc.named_scope("AG"):
            nc.gpsimd.collective_compute("AllGather", Alu.bypass, replica_groups=grp,
                                         ins=[pk_sh[:].opt()], outs=[pk_all[:].opt()])
            nc.gpsimd.collective_compute("AllGather", Alu.bypass, replica_groups=grp,
                                         ins=[xn_sh[:].opt()], outs=[xn_all[:].opt()])

        # ================= phases D-F in a second scope =================
        with tc.tile_pool(name="moe", bufs=1) as mp, \
             tc.tile_pool(name="moe_ps", bufs=1, space="PSUM") as mps:

            # ---- routing: per-expert compaction via sparse_gather ----
            rt_ctx = nc.named_scope("Routing")
            rt_ctx.__enter__()
            ids_e = []    # [EPC][2] I32 [128, 1] token ids per slot chunk
            gates_e = []  # [EPC][2] F32 [128, 1] gate per slot chunk
            for e in range(EPC):
                # gather this expert's 16 payload rows from the AG'd pk
                sg_in = mp.tile([16, 128], F32, tag=f"sgin{e}", name=f"sgin{e}")
                nc.gpsimd.indirect_dma_start(
                    out=sg_in[:], out_offset=None, in_=pk_all[:],
                    in_offset=bass.IndirectOffsetOnAxis(
                        ap=eidx_t[:, e:e + 1], axis=0),
                    bounds_check=NCORES * E * 2 - 1, oob_is_err=False)
                sg_out = mp.tile([16, 16], F32, tag=f"sgout{e}", name=f"sgout{e}")
                nf = tp.tile([1, 1], U32, tag=f"nf{e}", name=f"nf{e}")
                nc.gpsimd.sparse_gather(out=sg_out[:], in_=sg_in[:],
                                        num_found=nf[:])
                nff = tp.tile([1, 1], F32, tag=f"nff{e}", name=f"nff{e}")
                nc.vector.tensor_copy(out=nff[:], in_=nf[:])
                nfp = ps_misc.tile([128, 1], F32, space="PSUM", tag="misc",
                                   name=f"nfp{e}")
                nc.tensor.matmul(out=nfp[:], lhsT=ones_f[:], rhs=nff[:],
                                 start=True, stop=True)
                nfb = tp.tile([128, 1], F32, tag=f"nfb{e}", name=f"nfb{e}")
                nc.vector.tensor_copy(out=nfb[:], in_=nfp[:])
                # transpose -> linear slot order, roundtrip to [128, 1] columns
                ptc = mps.tile([16, 16], F32, space="PSUM", tag="tr2", bufs=2,
                               name="sgTp")
                nc.tensor.transpose(out=ptc[:], in_=sg_out[:],
                                    identity=identf[:16, :16])
                sgT = mp.tile([16, 16], F32, tag=f"sgT{e}", name=f"sgT{e}")
                nc.vector.tensor_copy(out=sgT[:], in_=ptc[:])
                nc.sync.dma_start(
                    out=sg_scr[e][:].rearrange("a b -> (a b)")
                    .rearrange("(p f) -> p f", p=16), in_=sgT[:])
                ids_i, gates_i = [], []
                for ct in range(2):
                    clv = tp.tile([128, 1], F32, tag=f"clv{e}_{ct}",
                                  name=f"clv{e}_{ct}")
                    nc.sync.dma_start(
                        out=clv[:],
                        in_=sg_scr[e][ct:ct + 1, :]
                        .rearrange("one r -> (one r)")
                        .rearrange("(r one) -> r one", one=1))
                    keep = tp.tile([128, 1], F32, tag=f"kp{e}_{ct}",
                                   name=f"kp{e}_{ct}")
                    nc.vector.tensor_tensor(out=keep[:], in0=nfb[:],
                                            in1=sidx_t[:, ct:ct + 1], op=Alu.is_gt)
                    nc.vector.tensor_scalar_add(out=clv[:], in0=clv[:], scalar1=1.0)
                    nc.vector.tensor_tensor(out=clv[:], in0=clv[:], in1=keep[:],
                                            op=Alu.mult)
                    nc.vector.tensor_scalar_add(out=clv[:], in0=clv[:], scalar1=-1.0)
                    # unpack id + gate (empty slots are -1 -> id 0, gate 0).
                    # floor via int-cast roundtrip; correct in case the cast
                    # rounds up instead of truncating.
                    nc.vector.tensor_scalar_max(out=clv[:], in0=clv[:], scalar1=0.0)
                    ids0 = tp.tile([128, 1], I32, tag=f"ids0{e}_{ct}",
                                   name=f"ids0{e}_{ct}")
                    nc.vector.tensor_copy(out=ids0[:], in_=clv[:])
                    idf = tp.tile([128, 1], F32, tag=f"idf{e}_{ct}",
                                  name=f"idf{e}_{ct}")
                    nc.vector.tensor_copy(out=idf[:], in_=ids0[:])
                    wrong = tp.tile([128, 1], F32, tag=f"wr{e}_{ct}",
                                    name=f"wr{e}_{ct}")
                    nc.vector.tensor_tensor(out=wrong[:], in0=idf[:], in1=clv[:],
                                            op=Alu.is_gt)
                    nc.vector.tensor_tensor(out=idf[:], in0=idf[:], in1=wrong[:],
                                            op=Alu.subtract)
                    gat = tp.tile([128, 1], F32, tag=f"gat{e}_{ct}",
                                  name=f"gat{e}_{ct}")
                    nc.vector.tensor_tensor(out=gat[:], in0=clv[:], in1=idf[:],
                                            op=Alu.subtract)
                    ids = tp.tile([128, 1], I32, tag=f"ids{e}_{ct}",
                                  name=f"ids{e}_{ct}")
                    nc.vector.tensor_copy(out=ids[:], in_=idf[:])
                    ids_i.append(ids)
                    gates_i.append(gat)
                ids_e.append(ids_i)
                gates_e.append(gates_i)
            rt_ctx.__exit__(None, None, None)

            # ---- expert compute ----
            for e in range(EPC):
                e_ctx = nc.named_scope(f"Expert{e}")
                e_ctx.__enter__()
                ids_i = ids_e[e]
                gates_i = gates_e[e]
                xeT = [mp.tile([128, CAP], BF16, tag=f"xeT_{kc}", bufs=2,
                               name=f"xeT{e}_{kc}") for kc in range(KC)]
                for ct in range(2):
                    xe = mp.tile([128, D], BF16, tag="xe", bufs=2, name=f"xe{e}_{ct}")
                    nc.gpsimd.indirect_dma_start(
                        out=xe[:], out_offset=None, in_=xn_all[:],
                        in_offset=bass.IndirectOffsetOnAxis(
                            ap=ids_i[ct][:, :1], axis=0),
                        bounds_check=NTOK - 1, oob_is_err=False)
                    for kc in range(KC):
                        pt = mps.tile([128, 128], BF16, space="PSUM", tag="tr2",
                                      bufs=2, name="xetp")
                        nc.tensor.transpose(out=pt[:],
                                            in_=xe[:, kc * 128:(kc + 1) * 128],
                                            identity=identb[:])
                        nc.vector.tensor_copy(out=xeT[kc][:, ct * 128:(ct + 1) * 128],
                                              in_=pt[:])
                wd_e = []
                for hc in range(HC):
                    t_ = wdp.tile([128, D], BF16, tag="wd", name=f"wd{e}_{hc}")
                    nc.sync.dma_start(out=t_[:],
                                      in_=wd_in[e, hc * 128:(hc + 1) * 128, :])
                    wd_e.append(t_)
                aT = []
                for q in range(QH):
                    wg_q, wu_q = [], []
                    for kc in range(KC):
                        tg = moew.tile([128, 512], BF16, tag="wgq",
                                       name=f"wg{e}_{q}_{kc}")
                        nc.gpsimd.dma_start(
                            out=tg[:], in_=wg_in[e, kc * 128:(kc + 1) * 128,
                                                 q * 512:(q + 1) * 512])
                        wg_q.append(tg)
                        tu = moew.tile([128, 512], BF16, tag="wuq",
                                       name=f"wu{e}_{q}_{kc}")
                        nc.gpsimd.dma_start(
                            out=tu[:], in_=wu_in[e, kc * 128:(kc + 1) * 128,
                                                 q * 512:(q + 1) * 512])
                        wu_q.append(tu)
                    for hcl in range(4):
                        hsl = slice(hcl * 128, (hcl + 1) * 128)
                        hT = mps.tile([128, CAP], F32, space="PSUM", tag="hu", bufs=3,
                                      name="hT")
                        for kc in range(KC):
                            nc.tensor.matmul(out=hT[:], lhsT=wg_q[kc][:, hsl],
                                             rhs=xeT[kc][:],
                                             start=(kc == 0), stop=(kc == KC - 1))
                        uT = mps.tile([128, CAP], F32, space="PSUM", tag="hu", bufs=3,
                                      name="uT")
                        for kc in range(KC):
                            nc.tensor.matmul(out=uT[:], lhsT=wu_q[kc][:, hsl],
                                             rhs=xeT[kc][:],
                                             start=(kc == 0), stop=(kc == KC - 1))
                        sl = tp.tile([128, CAP], BF16, tag="silu", bufs=2, name="silu")
                        if SIM_SAFE_SILU:
                            sgm = tp.tile([128, CAP], F32, tag="sgm", bufs=2,
                                          name="sgm")
                            nc.scalar.activation(out=sgm[:], in_=hT[:],
                                                 func=Act.Sigmoid)
                            nc.vector.tensor_tensor(out=sl[:], in0=sgm[:],
                                                    in1=hT[:], op=Alu.mult)
                        else:
                            nc.scalar.activation(out=sl[:], in_=hT[:], func=Act.Silu)
                        a_ = mp.tile([128, CAP], BF16, tag="aT", bufs=20,
                                     name=f"aT{e}_{q}_{hcl}")
                        nc.vector.tensor_tensor(out=a_[:], in0=sl[:], in1=uT[:],
                                                op=Alu.mult)
                        aT.append(a_)
                for ct in range(2):
                    y_p = mps.tile([128, D], F32, space="PSUM", tag="y", bufs=1,
                                   name="y_p")
                    for hc in range(HC):
                        nc.tensor.matmul(out=y_p[:],
                                         lhsT=aT[hc][:, ct * 128:(ct + 1) * 128],
                                         rhs=wd_e[hc][:], start=(hc == 0),
                                         stop=(hc == HC - 1))
                    y_g = tp.tile([128, D], BF16, tag="y_g", bufs=2, name=f"y_g{e}_{ct}")
                    nc.vector.tensor_scalar_mul(out=y_g[:], in0=y_p[:],
                                                scalar1=gates_i[ct][:, :1])
                    nc.gpsimd.indirect_dma_start(
                        out=accum[:], out_offset=bass.IndirectOffsetOnAxis(
                            ap=ids_i[ct][:, :1], axis=0),
                        in_=y_g[:], in_offset=None, compute_op=Alu.add,
                        bounds_check=NTOK - 1, oob_is_err=False)
                e_ctx.__exit__(None, None, None)

            # ---- reduce-scatter + residual ----
            with nc.named_scope("RS"):
                nc.gpsimd.collective_compute("ReduceScatter", Alu.add,
                                             replica_groups=grp,
                                             ins=[accum[:].opt()],
                                             outs=[rs_out[:].opt()])
            for qt in range(2):
                rs_t = tp.tile([128, D], BF16, tag=f"rs{qt}", name=f"rs{qt}")
                nc.sync.dma_start(out=rs_t[:], in_=rs_out[qt * 128:(qt + 1) * 128, :])
                o_t = tp.tile([128, D], F32, tag=f"ofin{qt}", name=f"ofin{qt}")
                nc.vector.tensor_tensor(out=o_t[:], in0=x2[qt][:], in1=rs_t[:],
                                        op=Alu.add)
                nc.sync.dma_start(out=out_dram[qt * 128:(qt + 1) * 128, :], in_=o_t[:])

    nc.compile()
    return nc


_NC_CACHE = None


def _get_program():
    global _NC_CACHE
    if _NC_CACHE is None:
        _NC_CACHE = build_program()
    return _NC_CACHE


def make_in_maps(x, enc_out, causal_mask, norm1_w, norm2_w, norm3_w,
                 sa_wq, sa_wk, sa_wv, sa_wo, ca_wq, ca_wk, ca_wv, ca_wo,
                 router_w, moe_wg, moe_wu, moe_wd):
    x = np.asarray(x, np.float32)
    enc_out = np.asarray(enc_out, np.float32)
    causal_mask = np.asarray(causal_mask)
    fullmask = np.where(causal_mask, np.float32(-1e30), np.float32(0.0))
    norms = np.stack([np.asarray(norm1_w, np.float32),
                      np.asarray(norm2_w, np.float32),
                      np.asarray(norm3_w, np.float32)], 0)
    bf = lambda a: np.asarray(a, np.float32).astype(ml_dtypes.bfloat16)
    shared = {
        "norms": norms,
        "router_w": np.asarray(router_w, np.float32),
        "sa_wq": bf(sa_wq), "sa_wk": bf(sa_wk),
        "sa_wv": bf(sa_wv), "sa_wo": bf(sa_wo),
        "ca_wq": bf(ca_wq), "ca_wk": bf(ca_wk),
        "ca_wv": bf(ca_wv), "ca_wo": bf(ca_wo),
    }
    moe_wg = np.asarray(moe_wg, np.float32).astype(ml_dtypes.bfloat16)
    moe_wu = np.asarray(moe_wu, np.float32).astype(ml_dtypes.bfloat16)
    moe_wd = np.asarray(moe_wd, np.float32).astype(ml_dtypes.bfloat16)

    in_maps = []
    for c in range(NCORES):
        b, h = c // 2, c % 2
        perm = np.concatenate([np.arange(h * S, (h + 1) * S),
                               np.arange((1 - h) * S, (2 - h) * S)])
        xb_perm = x[b][perm]
        mrows = fullmask[h * S:(h + 1) * S][:, perm]
        # rows of pk_all [NCORES*E*2, 128] holding this core's 3 experts
        eidx = np.empty((16, EPC), np.int32)
        for i in range(EPC):
            eg = EPC * c + i
            for p in range(16):
                eidx[p, i] = (p // 2) * E * 2 + eg * 2 + (p % 2)
        # global token ids for this core's two 128-token tiles
        gids = (np.float32(c * S)
                + np.arange(128, dtype=np.float32)[:, None]
                + np.float32(128) * np.arange(2, dtype=np.float32)[None, :])
        m = dict(shared)
        m["xb"] = np.ascontiguousarray(xb_perm)
        m["encb"] = np.ascontiguousarray(bf(enc_out[b]))
        m["maskadd"] = np.ascontiguousarray(mrows)
        m["eidx"] = eidx
        m["gids"] = np.ascontiguousarray(gids)
        m["sidx"] = np.ascontiguousarray(
            np.arange(128, dtype=np.float32)[:, None]
            + np.float32(128) * np.arange(2, dtype=np.float32)[None, :])
        m["wg"] = np.ascontiguousarray(moe_wg[EPC * c:EPC * (c + 1)])
        m["wu"] = np.ascontiguousarray(moe_wu[EPC * c:EPC * (c + 1)])
        m["wd"] = np.ascontiguousarray(moe_wd[EPC * c:EPC * (c + 1)])
        in_maps.append(m)
    return in_maps


def assemble_out(results):
    out = np.empty((B, T, D), np.float32)
    for c in range(NCORES):
        b, h = c // 2, c % 2
        out[b, h * S:(h + 1) * S] = results[c]["out"]
    return out


def kernel(**inputs):
    nc = _get_program()
    in_maps = make_in_maps(**inputs)
    res = run_bass_kernel_spmd(nc, in_maps, list(range(NCORES)))
    return assemble_out(res.results)


if __name__ == "__main__":
    import reference
    inp = reference.setup_inputs()
    got = kernel(**{k: np.asarray(v) for k, v in inp.items()})
    exp = np.asarray(reference.reference(**inp))
    err = np.abs(got - exp)
    print("abs max err:", err.max(), "rel:", err.max() / np.abs(exp).max())

